# revision 1
# baseline (speedup 1.0000x reference)
"""Dense transformer block (rmsnorm+causal attention+rope / rmsnorm+SwiGLU) on 8 TRN2 cores.

Sharding:
  core j (j=0..7): batch b = j//4, head-group hg = j%4 (heads 4*hg..4*hg+3).
  Phase A (attention) is head-sharded: each core computes rmsnorm(x[b]) -> QKV for
  its 4 heads -> rope -> causal attention -> O^T [512, T].
  Two 8-core AllToAlls (heads {0,1} then {2,3}) reshard to row-sharding; the first
  overlaps the second half of attention, the second overlaps the first half of the
  output projection. Cross-batch slices are neutralized by zero rows in a
  per-core-padded w_proj.
  Phase B (proj residual + rmsnorm2 + SwiGLU MLP) is row-sharded: each core
  computes its 512 rows against full weights; outputs are concatenated on host.

Matmul operands are bf16 (weights pre-cast on host, w_norm folded into weight
rows); statistics, softmax denominators, residual stream and PSUM stay fp32.
"""

import numpy as np
import ml_dtypes

import concourse.bass as bass
import concourse.mybir as mybir
import concourse.tile as tile
from concourse import bacc
from concourse import bass_utils
from concourse.masks import make_identity

AF = mybir.ActivationFunctionType
ALU = mybir.AluOpType
F32 = mybir.dt.float32
BF16 = mybir.dt.bfloat16
MMDT = BF16
NP_MMDT = ml_dtypes.bfloat16

P = 128
T = 2048
C = 2048
D = 128
H = 16
HPC = 4          # heads per core
HID = 5632
HID_T = HID // P  # 44 hid tiles
TQ = 512         # q-chunk / output row-block per core
EPS = 1e-6
ROPE_BASE = 10000.0
CT = C // P      # 16 contraction tiles
QKV_CHUNK = 1024


def _build():
    nc = bacc.Bacc(None, target_bir_lowering=False, num_devices=8)

    # ---- kernel I/O ----
    x_full = nc.dram_tensor("x_full", [T, C], F32, kind="ExternalInput")
    x_t = nc.dram_tensor("x_t", [C, T], F32, kind="ExternalInput")
    x_rows = nc.dram_tensor("x_rows", [TQ, C], F32, kind="ExternalInput")
    wq = nc.dram_tensor("wq", [C, HPC * D], MMDT, kind="ExternalInput")
    wk = nc.dram_tensor("wk", [C, HPC * D], MMDT, kind="ExternalInput")
    wv = nc.dram_tensor("wv", [C, HPC * D], MMDT, kind="ExternalInput")
    wpe = nc.dram_tensor("wpe", [2 * C, C], MMDT, kind="ExternalInput")
    w1t = nc.dram_tensor("w1t", [HID_T, C, P], MMDT, kind="ExternalInput")
    w2t = nc.dram_tensor("w2t", [HID_T, C, P], MMDT, kind="ExternalInput")
    w3 = nc.dram_tensor("w3", [HID, C], MMDT, kind="ExternalInput")
    rope_t = nc.dram_tensor("rope_t", [D, T], F32, kind="ExternalInput")
    tri = nc.dram_tensor("tri", [P, P], MMDT, kind="ExternalInput")
    out = nc.dram_tensor("out", [TQ, C], F32, kind="ExternalOutput")

    inv_sqrt_d = 1.0 / float(np.sqrt(D))

    with tile.TileContext(nc) as tc:
        with (
            tc.tile_pool(name="const", bufs=1) as const,
            tc.tile_pool(name="dram", bufs=1, space="DRAM") as dram,
        ):
            # ---- constants ----
            ident_f = const.tile([P, P], F32)
            make_identity(nc, ident_f)
            ident = const.tile([P, P], MMDT)
            nc.vector.tensor_copy(out=ident, in_=ident_f)
            ones_f = const.tile([P, 1], F32)
            nc.vector.memset(ones_f, 1.0)
            ones_r = const.tile([P, 1], MMDT)
            nc.vector.tensor_copy(out=ones_r, in_=ones_f)
            eps_sb = const.tile([P, 1], F32)
            nc.vector.memset(eps_sb, EPS)
            rope_sb = const.tile([D, T], F32)
            nc.sync.dma_start(out=rope_sb, in_=rope_t[:, :])
            tri_sb = const.tile([P, P], MMDT)
            nc.sync.dma_start(out=tri_sb, in_=tri[:, :])
            rstd_T = const.tile([1, T], F32)

            # ---- DRAM scratch ----
            qT_d = dram.tile([HPC * D, T], MMDT)
            kT_d = dram.tile([HPC * D, T], MMDT)
            v_d = dram.tile([T, HPC * D], MMDT)
            xmid_d = dram.tile([TQ, C], F32)
            a2a1_in = dram.tile([8, 2 * P, TQ], MMDT)
            a2a1_out = dram.tile([8, 2 * P, TQ], MMDT)
            a2a2_in = dram.tile([8, 2 * P, TQ], MMDT)
            a2a2_out = dram.tile([8, 2 * P, TQ], MMDT)

            # ================= Phase A1+A2: rmsnorm1 + h^T + QKV =================
            with (
                tc.tile_pool(name="p12", bufs=2) as p12,
                tc.tile_pool(name="p12psum", bufs=2, space="PSUM") as pp12,
            ):
                CHUNKS = [256, 256, 512, 1024]
                t0 = 0
                for ch, CHW in enumerate(CHUNKS):
                    rt_per_chunk = CHW // P
                    SUBW = min(TQ, CHW)
                    # rstd for this chunk's rows -> rstd_T[0, t0:t0+chunk] (via PE transpose)
                    for rt in range(rt_per_chunk):
                        row0 = t0 + rt * P
                        xt = p12.tile([P, C], F32, tag="xt", bufs=2)
                        nc.sync.dma_start(out=xt, in_=x_full[row0 : row0 + P, :])
                        sq = p12.tile([P, C], F32, tag="sq", bufs=1)
                        ssum = p12.tile([P, 1], F32, tag="ssum", bufs=3)
                        nc.scalar.activation(sq, xt, AF.Square, accum_out=ssum)
                        rstd = p12.tile([P, 1], F32, tag="rstd", bufs=3)
                        nc.scalar.activation(rstd, ssum, AF.Sqrt, bias=eps_sb, scale=1.0 / C)
                        nc.vector.reciprocal(out=rstd, in_=rstd)
                        prs = pp12.tile([1, P], F32, tag="rsT", bufs=2)
                        nc.tensor.matmul(prs, rstd, ident_f, start=True, stop=True)
                        nc.scalar.activation(rstd_T[0:1, row0 : row0 + P], prs, AF.Copy)

                    rstd_bc = p12.tile([P, QKV_CHUNK], F32, tag="rstd_bc", bufs=2, name="rstd_bc")[
                        :, :CHW
                    ]
                    nc.gpsimd.partition_broadcast(
                        rstd_bc[:], rstd_T[0:1, t0 : t0 + CHW]
                    )
                    hT = p12.tile([P, CT, QKV_CHUNK], MMDT, tag="hT", bufs=2, name="hT")[:, :, :CHW]
                    for ct in range(CT):
                        xtt = p12.tile([P, QKV_CHUNK], F32, tag="xtt", bufs=3, name="xtt")[:, :CHW]
                        nc.sync.dma_start(
                            out=xtt, in_=x_t[ct * P : (ct + 1) * P, t0 : t0 + CHW]
                        )
                        nc.vector.tensor_tensor(
                            out=hT[:, ct, :], in0=xtt, in1=rstd_bc, op=ALU.mult
                        )

                    # q^T / k^T with fused rope on eviction
                    for which, w_in, dst in (("q", wq, qT_d), ("k", wk, kT_d)):
                        for m in range(HPC):
                            wt = p12.tile([P, CT, P], MMDT, tag="wt", bufs=3)
                            nc.sync.dma_start(
                                out=wt,
                                in_=w_in[:, m * P : (m + 1) * P].rearrange(
                                    "(ct p) d -> p ct d", p=P
                                ),
                            )
                            for sub in range(CHW // SUBW):
                                s0 = sub * SUBW
                                g0 = t0 + s0
                                pq = pp12.tile([P, TQ], F32, tag="qk", bufs=3, name="pq")[:, :SUBW]
                                for ct in range(CT):
                                    nc.tensor.matmul(
                                        pq,
                                        wt[:, ct, :],
                                        hT[:, ct, s0 : s0 + SUBW],
                                        start=(ct == 0),
                                        stop=(ct == CT - 1),
                                    )
                                # rope: rows 0:64 = x1*cos - x2*sin ; 64:128 = x1*sin + x2*cos
                                HD2 = D // 2
                                x1 = pq[0:HD2, :]
                                x2 = pq[HD2:P, :]
                                cosw = rope_sb[0:HD2, g0 : g0 + SUBW]
                                sinw = rope_sb[HD2:D, g0 : g0 + SUBW]
                                rop = p12.tile([P, TQ], MMDT, tag="rope_out", bufs=4, name="rop")[
                                    :, :SUBW
                                ]
                                tm1 = p12.tile([HD2, TQ], F32, tag="tm1", bufs=2, name="tm1")[:, :SUBW]
                                tm2 = p12.tile([HD2, TQ], F32, tag="tm2", bufs=2, name="tm2")[:, :SUBW]
                                nc.vector.tensor_tensor(out=tm1, in0=x1, in1=cosw, op=ALU.mult)
                                nc.vector.tensor_tensor(out=tm2, in0=x2, in1=sinw, op=ALU.mult)
                                nc.vector.tensor_tensor(
                                    out=rop[0:HD2, :], in0=tm1, in1=tm2, op=ALU.subtract
                                )
                                nc.vector.tensor_tensor(out=tm1, in0=x1, in1=sinw, op=ALU.mult)
                                nc.vector.tensor_tensor(out=tm2, in0=x2, in1=cosw, op=ALU.mult)
                                nc.vector.tensor_tensor(
                                    out=rop[HD2:P, :], in0=tm1, in1=tm2, op=ALU.add
                                )
                                nc.sync.dma_start(
                                    out=dst[m * P : (m + 1) * P, g0 : g0 + SUBW], in_=rop
                                )

                    # v in row layout [T, HPC*D]
                    wv_sb = p12.tile([P, CT, HPC * D], MMDT, tag="wv_sb", bufs=1)
                    nc.sync.dma_start(
                        out=wv_sb, in_=wv.rearrange("(ct p) d -> p ct d", p=P)
                    )
                    for rt in range(rt_per_chunk):
                        pv = pp12.tile([P, HPC * D], F32, tag="v", bufs=3)
                        for ct in range(CT):
                            nc.tensor.matmul(
                                pv,
                                hT[:, ct, rt * P : (rt + 1) * P],
                                wv_sb[:, ct, :],
                                start=(ct == 0),
                                stop=(ct == CT - 1),
                            )
                        vt = p12.tile([P, HPC * D], MMDT, tag="vt", bufs=3)
                        nc.scalar.activation(vt, pv, AF.Copy)
                        nc.sync.dma_start(
                            out=v_d[t0 + rt * P : t0 + (rt + 1) * P, :], in_=vt
                        )
                    t0 += CHW

            # ================= Phase A3: causal attention (+ split A2A) ==========
            wpre_ctx = tc.tile_pool(name="wpre", bufs=1)
            wpre = wpre_ctx.__enter__()
            wpe_pre = wpre.tile([P, 2, 16, TQ], MMDT, tag="wpe_pre", bufs=1)
            lp0 = wpre.tile([P, 8, 2, TQ], MMDT, tag="lp0", bufs=1)
            lp1 = wpre.tile([P, 8, 2, TQ], MMDT, tag="lp1", bufs=1)
            with (
                tc.tile_pool(name="att", bufs=2) as att,
                tc.tile_pool(name="attpsum", bufs=2, space="PSUM") as pat,
            ):
                wpe_pre_ctr = [0]

                def _drip_wpe(n):
                    while wpe_pre_ctr[0] < 32 and n > 0:
                        i = wpe_pre_ctr[0]
                        cc_, sa_ = i // 16, i % 16
                        nc.sync.dma_start(
                            out=wpe_pre[:, cc_, sa_, :],
                            in_=wpe[sa_ * P : (sa_ + 1) * P, cc_ * TQ : (cc_ + 1) * TQ],
                        )
                        wpe_pre_ctr[0] += 1
                        n -= 1

                for h in range(HPC):
                    a2a_in = a2a1_in if h < 2 else a2a2_in
                    hrow0 = (h % 2) * P
                    kT_h = att.tile([P, T], MMDT, tag="kT_h", bufs=2)
                    nc.sync.dma_start(out=kT_h, in_=kT_d[h * P : (h + 1) * P, :])
                    v_h = att.tile([P, T // P, D], MMDT, tag="v_h", bufs=2)
                    nc.sync.dma_start(
                        out=v_h,
                        in_=v_d[:, h * P : (h + 1) * P].rearrange(
                            "(kb p) d -> p kb d", p=P
                        ),
                    )
                    for qc in range(T // TQ):
                        if h == 3 and qc == 3:
                            for s_ in range(8):
                                for a_ in range(2):
                                    nc.sync.dma_start(
                                        out=lp0[:, s_, a_, :],
                                        in_=a2a1_out[s_, a_ * P : (a_ + 1) * P, :],
                                    )
                        qT_c = att.tile([P, TQ], MMDT, tag="qT_c", bufs=3)
                        nc.sync.dma_start(
                            out=qT_c,
                            in_=qT_d[h * P : (h + 1) * P, qc * TQ : (qc + 1) * TQ],
                        )
                        nkb = 4 * qc + 4
                        l_ps = pat.tile([1, TQ], F32, tag="l", bufs=2)
                        o_ps = pat.tile([P, TQ], F32, tag="o", bufs=2)
                        es = []
                        for kb in range(nkb):
                            r = kb - 4 * qc
                            q0 = max(0, r * P)
                            st = pat.tile([P, TQ], F32, tag="st", bufs=4)
                            nc.tensor.matmul(
                                st[:, q0:TQ],
                                kT_h[:, kb * P : (kb + 1) * P],
                                qT_c[:, q0:TQ],
                                start=True,
                                stop=True,
                            )
                            e = att.tile([P, TQ], MMDT, tag="e", bufs=18)
                            nc.scalar.activation(
                                e[:, q0:TQ], st[:, q0:TQ], AF.Exp, scale=inv_sqrt_d
                            )
                            if r >= 0:
                                nc.vector.tensor_tensor(
                                    out=e[:, q0 : q0 + P],
                                    in0=e[:, q0 : q0 + P],
                                    in1=tri_sb,
                                    op=ALU.mult,
                                )
                            es.append((e, q0))
                        for kb in range(nkb):
                            e, q0 = es[kb]
                            nc.tensor.matmul(
                                l_ps[:, q0:TQ],
                                ones_r,
                                e[:, q0:TQ],
                                start=(kb == 0),
                                stop=(kb == nkb - 1),
                            )
                            nc.tensor.matmul(
                                o_ps[:, q0:TQ],
                                v_h[:, kb, :],
                                e[:, q0:TQ],
                                start=(kb == 0),
                                stop=(kb == nkb - 1),
                            )
                        l_inv = att.tile([1, TQ], F32, tag="l_inv", bufs=2)
                        nc.vector.reciprocal(out=l_inv, in_=l_ps)
                        l_bc = att.tile([P, TQ], F32, tag="l_bc", bufs=2)
                        nc.gpsimd.partition_broadcast(l_bc[:], l_inv[:])
                        oT = att.tile([P, TQ], MMDT, tag="oT", bufs=3)
                        nc.vector.tensor_tensor(out=oT, in0=o_ps, in1=l_bc, op=ALU.mult)
                        nc.sync.dma_start(out=a2a_in[qc, hrow0 : hrow0 + P, :], in_=oT)
                        nc.sync.dma_start(
                            out=a2a_in[qc + 4, hrow0 : hrow0 + P, :], in_=oT
                        )
                        _drip_wpe(2)
                    if h == 1:
                        nc.gpsimd.collective_compute(
                            "AllToAll",
                            ALU.bypass,
                            replica_groups=[[0, 1, 2, 3, 4, 5, 6, 7]],
                            ins=[a2a1_in.opt()],
                            outs=[a2a1_out.opt()],
                        )
                    if h == 3:
                        nc.gpsimd.collective_compute(
                            "AllToAll",
                            ALU.bypass,
                            replica_groups=[[0, 1, 2, 3, 4, 5, 6, 7]],
                            ins=[a2a2_in.opt()],
                            outs=[a2a2_out.opt()],
                        )

            # ================= Phase B1: proj + residual (two-stage) =============
            with (
                tc.tile_pool(name="proj", bufs=2) as prj,
                tc.tile_pool(name="projpsum", bufs=2, space="PSUM") as ppj,
            ):
                xr = prj.tile([P, 4, C], F32, tag="xr", bufs=1)
                nc.sync.dma_start(
                    out=xr, in_=x_rows.rearrange("(qt p) c -> p qt c", p=P)
                )
                y0acc = prj.tile([P, 4, C], F32, tag="y0acc", bufs=1)
                for half, a2a_o in ((0, a2a1_out), (1, a2a2_out)):
                    lp = lp0 if half == 0 else lp1
                    if half == 1:
                        for s_ in range(8):
                            for a_ in range(2):
                                nc.sync.dma_start(
                                    out=lp[:, s_, a_, :],
                                    in_=a2a_o[s_, a_ * P : (a_ + 1) * P, :],
                                )
                    for cc in range(4):
                        yps = [
                            ppj.tile([P, TQ], F32, tag=f"y{qt}", bufs=1, name=f"y{qt}")
                            for qt in range(4)
                        ]
                        for s in range(8):
                            for a in range(2):
                                if half == 0 and cc < 2:
                                    wt = wpe_pre[:, cc, s * 2 + a, :]
                                else:
                                    wt = prj.tile([P, TQ], MMDT, tag="wpe_t", bufs=6)
                                    nc.sync.dma_start(
                                        out=wt,
                                        in_=wpe[
                                            (half * 16 + s * 2 + a) * P
                                            : (half * 16 + s * 2 + a + 1) * P,
                                            cc * TQ : (cc + 1) * TQ,
                                        ],
                                    )
                                for qt in range(4):
                                    nc.tensor.matmul(
                                        yps[qt],
                                        lp[:, s, a, qt * P : (qt + 1) * P],
                                        wt,
                                        start=(s == 0 and a == 0),
                                        stop=(s == 7 and a == 1),
                                    )
                        for qt in range(4):
                            if half == 0:
                                nc.scalar.activation(
                                    y0acc[:, qt, cc * TQ : (cc + 1) * TQ],
                                    yps[qt],
                                    AF.Copy,
                                )
                            else:
                                t1 = prj.tile([P, TQ], F32, tag="t1", bufs=3)
                                nc.vector.tensor_tensor(
                                    out=t1,
                                    in0=yps[qt],
                                    in1=y0acc[:, qt, cc * TQ : (cc + 1) * TQ],
                                    op=ALU.add,
                                )
                                xm = prj.tile([P, TQ], F32, tag="xm", bufs=3)
                                nc.vector.tensor_tensor(
                                    out=xm,
                                    in0=t1,
                                    in1=xr[:, qt, cc * TQ : (cc + 1) * TQ],
                                    op=ALU.add,
                                )
                                nc.sync.dma_start(
                                    out=xmid_d[
                                        qt * P : (qt + 1) * P, cc * TQ : (cc + 1) * TQ
                                    ],
                                    in_=xm,
                                )

            wpre_ctx.__exit__(None, None, None)

            # ================= Phase B2: rmsnorm2 + h2^T + SwiGLU ================
            with tc.tile_pool(name="mlp", bufs=2) as mlp:
                h2T = mlp.tile([P, CT, TQ], MMDT, tag="h2T", bufs=1)
                with tc.tile_pool(name="pml_tr", bufs=2, space="PSUM") as pml_tr:
                    for rt in range(TQ // P):
                        xt = mlp.tile([P, C], F32, tag="xt2", bufs=2)
                        nc.sync.dma_start(out=xt, in_=xmid_d[rt * P : (rt + 1) * P, :])
                        sq = mlp.tile([P, C], F32, tag="sq2", bufs=1)
                        ssum = mlp.tile([P, 1], F32, tag="ssum2", bufs=2)
                        nc.scalar.activation(sq, xt, AF.Square, accum_out=ssum)
                        rstd = mlp.tile([P, 1], F32, tag="rstd2", bufs=2)
                        nc.scalar.activation(rstd, ssum, AF.Sqrt, bias=eps_sb, scale=1.0 / C)
                        nc.vector.reciprocal(out=rstd, in_=rstd)
                        hrow = mlp.tile([P, C], MMDT, tag="hrow2", bufs=2)
                        nc.vector.tensor_scalar(
                            out=hrow, in0=xt, scalar1=rstd, scalar2=None, op0=ALU.mult
                        )
                        for ct in range(CT):
                            ptr = pml_tr.tile([P, P], MMDT, tag="tr2", bufs=4)
                            nc.tensor.transpose(ptr, hrow[:, ct * P : (ct + 1) * P], ident)
                            nc.scalar.activation(
                                h2T[:, ct, rt * P : (rt + 1) * P], ptr, AF.Copy
                            )

                # ---- SwiGLU ----
                pml = ctx_pml = tc.tile_pool(name="pml_mm", bufs=2, space="PSUM")
                pml = pml.__enter__()
                HHALF = HID_T // 4  # 11
                NPART = 4
                y3acc = mlp.tile([P, 4, C], F32, tag="y3acc", bufs=1)
                for half in range(NPART):
                    uT = mlp.tile([P, HHALF, TQ], MMDT, tag="uT", bufs=2)
                    for ht in range(HHALF):
                        htg = half * HHALF + ht
                        w1_sb = mlp.tile([P, CT, P], MMDT, tag="w1_sb", bufs=3)
                        nc.sync.dma_start(
                            out=w1_sb,
                            in_=w1t[htg].rearrange("(ct p) d -> p ct d", p=P),
                        )
                        w2_sb = mlp.tile([P, CT, P], MMDT, tag="w2_sb", bufs=3)
                        nc.sync.dma_start(
                            out=w2_sb,
                            in_=w2t[htg].rearrange("(ct p) d -> p ct d", p=P),
                        )
                        g1 = pml.tile([P, TQ], F32, tag="g1", bufs=2)
                        g2 = pml.tile([P, TQ], F32, tag="g2", bufs=2)
                        for ct in range(CT):
                            nc.tensor.matmul(
                                g1,
                                w1_sb[:, ct, :],
                                h2T[:, ct, :],
                                start=(ct == 0),
                                stop=(ct == CT - 1),
                            )
                        for ct in range(CT):
                            nc.tensor.matmul(
                                g2,
                                w2_sb[:, ct, :],
                                h2T[:, ct, :],
                                start=(ct == 0),
                                stop=(ct == CT - 1),
                            )
                        sil = mlp.tile([P, TQ], F32, tag="sil", bufs=3)
                        nc.scalar.activation(sil, g1, AF.Silu)
                        nc.vector.tensor_tensor(
                            out=uT[:, ht, :], in0=g2, in1=sil, op=ALU.mult
                        )
                    for cc in range(4):
                        y3ps = [
                            pml.tile(
                                [P, TQ], F32, tag=f"y3_{rt}", bufs=1, name=f"y3_{rt}"
                            )
                            for rt in range(4)
                        ]
                        for ht in range(HHALF):
                            htg = half * HHALF + ht
                            w3_sb = mlp.tile([P, TQ], MMDT, tag="w3_sb", bufs=4)
                            nc.sync.dma_start(
                                out=w3_sb,
                                in_=w3[htg * P : (htg + 1) * P, cc * TQ : (cc + 1) * TQ],
                            )
                            for rt in range(4):
                                nc.tensor.matmul(
                                    y3ps[rt],
                                    uT[:, ht, rt * P : (rt + 1) * P],
                                    w3_sb,
                                    start=(ht == 0),
                                    stop=(ht == HHALF - 1),
                                )
                        for rt in range(4):
                            if half == 0:
                                nc.scalar.activation(
                                    y3acc[:, rt, cc * TQ : (cc + 1) * TQ],
                                    y3ps[rt],
                                    AF.Copy,
                                )
                            elif half < NPART - 1:
                                nc.vector.tensor_tensor(
                                    out=y3acc[:, rt, cc * TQ : (cc + 1) * TQ],
                                    in0=y3ps[rt],
                                    in1=y3acc[:, rt, cc * TQ : (cc + 1) * TQ],
                                    op=ALU.add,
                                )
                            else:
                                xmt = mlp.tile([P, TQ], F32, tag="xmt", bufs=3)
                                nc.sync.dma_start(
                                    out=xmt,
                                    in_=xmid_d[
                                        rt * P : (rt + 1) * P, cc * TQ : (cc + 1) * TQ
                                    ],
                                )
                                osum = mlp.tile([P, TQ], F32, tag="osum", bufs=3)
                                nc.vector.tensor_tensor(
                                    out=osum,
                                    in0=y3ps[rt],
                                    in1=y3acc[:, rt, cc * TQ : (cc + 1) * TQ],
                                    op=ALU.add,
                                )
                                ofin = mlp.tile([P, TQ], F32, tag="ofin", bufs=3)
                                nc.vector.tensor_tensor(
                                    out=ofin, in0=osum, in1=xmt, op=ALU.add
                                )
                                nc.sync.dma_start(
                                    out=out[
                                        rt * P : (rt + 1) * P, cc * TQ : (cc + 1) * TQ
                                    ],
                                    in_=ofin,
                                )
                ctx_pml.__exit__(None, None, None)

    nc.compile()
    return nc


_NC_CACHE = None


def _get_nc():
    global _NC_CACHE
    if _NC_CACHE is None:
        _NC_CACHE = _build()
    return _NC_CACHE


def _host_inputs(x, w_norm1, w_qkv, w_proj, w_norm2, w1, w2, w3):
    x = np.asarray(x, dtype=np.float32)
    w_qkv = np.asarray(w_qkv, dtype=np.float32)
    w_proj = np.asarray(w_proj, dtype=np.float32)
    w_norm1 = np.asarray(w_norm1, dtype=np.float32)
    w_norm2 = np.asarray(w_norm2, dtype=np.float32)
    w1 = np.asarray(w1, dtype=np.float32)
    w2 = np.asarray(w2, dtype=np.float32)
    w3 = np.asarray(w3, dtype=np.float32)

    half = D // 2
    inv_freq = 1.0 / (ROPE_BASE ** (np.arange(half, dtype=np.float32) / half))
    pos = np.arange(T, dtype=np.float32)
    freqs = pos[:, None] * inv_freq[None, :]
    rope_tab = np.ascontiguousarray(
        np.concatenate([np.cos(freqs).T, np.sin(freqs).T], axis=0).astype(np.float32)
    )

    ql = np.arange(P)[None, :]
    kv = np.arange(P)[:, None]
    tri = (ql >= kv).astype(NP_MMDT)

    # fold w_norm into weight rows (h @ W == (x*rstd) @ (diag(wn) W))
    w_qkv_n = w_qkv * w_norm1[:, None]
    w1_n = w1 * w_norm2[:, None]
    w2_n = w2 * w_norm2[:, None]

    w1t = np.ascontiguousarray(
        w1_n.reshape(C, HID_T, P).transpose(1, 0, 2)
    ).astype(NP_MMDT)
    w2t = np.ascontiguousarray(
        w2_n.reshape(C, HID_T, P).transpose(1, 0, 2)
    ).astype(NP_MMDT)
    w3_b = w3.astype(NP_MMDT)

    in_maps = []
    for j in range(8):
        b, hg = j // 4, j % 4
        col0 = hg * HPC * D
        # wpe rows: [half(2), sender p(8), a(2), d(128)]; sender p's head = 4*(p%4)+half*2+a
        wpe = np.zeros((2 * C, C), dtype=NP_MMDT)
        for hf in range(2):
            for p_ in range(8):
                if p_ // 4 != b:
                    continue
                for a in range(2):
                    gh = 4 * (p_ % 4) + hf * 2 + a
                    dst = (hf * 16 + p_ * 2 + a) * P
                    wpe[dst : dst + P, :] = w_proj[gh * P : (gh + 1) * P, :].astype(
                        NP_MMDT
                    )
        xb = np.ascontiguousarray(x[b])
        in_maps.append(
            {
                "x_full": xb,
                "x_t": np.ascontiguousarray(xb.T),
                "x_rows": np.ascontiguousarray(xb[hg * TQ : (hg + 1) * TQ]),
                "wq": np.ascontiguousarray(
                    w_qkv_n[:, col0 : col0 + HPC * D]
                ).astype(NP_MMDT),
                "wk": np.ascontiguousarray(
                    w_qkv_n[:, C + col0 : C + col0 + HPC * D]
                ).astype(NP_MMDT),
                "wv": np.ascontiguousarray(
                    w_qkv_n[:, 2 * C + col0 : 2 * C + col0 + HPC * D]
                ).astype(NP_MMDT),
                "wpe": wpe,
                "w1t": w1t,
                "w2t": w2t,
                "w3": w3_b,
                "rope_t": rope_tab,
                "tri": tri,
            }
        )
    return in_maps


def kernel(x, w_norm1, w_qkv, w_proj, w_norm2, w1, w2, w3, _trace=False, _tmpdir=None):
    nc = _get_nc()
    in_maps = _host_inputs(x, w_norm1, w_qkv, w_proj, w_norm2, w1, w2, w3)
    kwargs = {}
    if _trace:
        kwargs = {"trace": True, "tmpdir": _tmpdir}
    res = bass_utils.run_bass_kernel_spmd(
        nc, in_maps, core_ids=list(range(8)), **kwargs
    )
    out = np.empty((2, T, C), dtype=np.float32)
    for j in range(8):
        out[j // 4, (j % 4) * TQ : (j % 4 + 1) * TQ, :] = res.results[j]["out"]
    kernel._last_exec_time_ns = res.exec_time_ns
    return out



# revision 5
# speedup vs baseline: 1.2299x; 1.2299x over previous
"""Dense transformer block (rmsnorm+causal attention+rope / rmsnorm+SwiGLU) on 8 TRN2 cores.

Sharding (v2):
  core j owns head pair {2j, 2j+1} for BOTH batches (attention head-parallel),
  and owns output row-chunk (b=j//4, rows (j%4)*512..) for phase B (row-parallel).

  Phase A: rmsnorm1 (rstd via all-ones matmul on x^T) -> h^T chunks -> QKV for the
  2 heads x 2 batches -> rope -> causal attention entirely in SBUF.
  Two 8-core AllToAlls reshard head->row: A2A#1 carries head 2j (fires at 50% of
  attention), A2A#2 carries head 2j+1 (fires at the end). Each slice r=(b*4+qc)
  is [128, 512] -> every byte is useful (no cross-batch padding).

  Phase B runs fully transposed (c on partitions, tokens on free dim):
  proj y^T accumulated per 128-col block of C over 16 received head blocks,
  residual from x^T slice, rmsnorm2 via all-ones matmul, SwiGLU with u^T kept
  for all 44 hidden tiles, w3 pass accumulates y3^T per c-block in PSUM.
  Output is written transposed [C, 512]; the host transposes back.

Matmul operands bf16 (weights pre-cast, w_norm folded); stats/PSUM fp32.
"""

import numpy as np
import ml_dtypes

import concourse.bass as bass
import concourse.mybir as mybir
import concourse.tile as tile
from concourse import bacc
from concourse import bass_utils

AF = mybir.ActivationFunctionType
ALU = mybir.AluOpType
F32 = mybir.dt.float32
BF16 = mybir.dt.bfloat16
MMDT = BF16
NP_MMDT = ml_dtypes.bfloat16

P = 128
T = 2048
C = 2048
D = 128
HD2 = D // 2
H = 16
HPC = 2          # heads per core
B = 2
HID = 5632
HID_T = HID // P  # 44
TQ = 512
CT = C // P      # 16
CHW = 512        # token chunk width in phase A
EPS = 1e-6
ROPE_BASE = 10000.0


def _build():
    nc = bacc.Bacc(None, target_bir_lowering=False, num_devices=8)

    # ---- kernel I/O ----
    x_tb = nc.dram_tensor("x_tb", [B, C, T], MMDT, kind="ExternalInput")
    x_res = nc.dram_tensor("x_res", [C, TQ], F32, kind="ExternalInput")
    wq = nc.dram_tensor("wq", [C, HPC * D], MMDT, kind="ExternalInput")
    wk = nc.dram_tensor("wk", [C, HPC * D], MMDT, kind="ExternalInput")
    wv = nc.dram_tensor("wv", [C, HPC * D], MMDT, kind="ExternalInput")
    wpe = nc.dram_tensor("wpe", [C, C], MMDT, kind="ExternalInput")
    w1t = nc.dram_tensor("w1t", [HID_T, C, P], MMDT, kind="ExternalInput")
    w2t = nc.dram_tensor("w2t", [HID_T, C, P], MMDT, kind="ExternalInput")
    w3 = nc.dram_tensor("w3", [HID, C], MMDT, kind="ExternalInput")
    rope_t = nc.dram_tensor("rope_t", [D, T], F32, kind="ExternalInput")
    tri = nc.dram_tensor("tri", [P, P], MMDT, kind="ExternalInput")
    out_t = nc.dram_tensor("out_t", [C, TQ], F32, kind="ExternalOutput")

    inv_sqrt_d = 1.0 / float(np.sqrt(D))

    with tile.TileContext(nc) as tc:
        with (
            tc.tile_pool(name="const", bufs=1) as const,
            tc.tile_pool(name="dram", bufs=1, space="DRAM") as dram,
        ):
            # ---- constants ----
            ones_f = const.tile([P, P], F32)
            nc.vector.memset(ones_f, 1.0)
            ones128 = const.tile([P, P], MMDT)
            nc.vector.tensor_copy(out=ones128, in_=ones_f)
            ones1 = const.tile([P, 1], MMDT)
            nc.vector.tensor_copy(out=ones1, in_=ones_f[:, 0:1])
            eps_sb = const.tile([P, 1], F32)
            nc.vector.memset(eps_sb, EPS)
            rope_sb = const.tile([D, T], F32)
            nc.sync.dma_start(out=rope_sb, in_=rope_t[:, :])
            tri_sb = const.tile([P, P], MMDT)
            nc.sync.dma_start(out=tri_sb, in_=tri[:, :])

            # ---- DRAM scratch for collectives ----
            a2a1_in = dram.tile([8, P, TQ], MMDT)
            a2a1_out = dram.tile([8, P, TQ], MMDT)
            a2a2_in = dram.tile([8, P, TQ], MMDT)
            a2a2_out = dram.tile([8, P, TQ], MMDT)

            # ================= Phase A: QKV + attention =================
            pers_ctx = tc.tile_pool(name="pers", bufs=1)
            pers = pers_ctx.__enter__()
            qT_sb = pers.tile([P, HPC, B, T], MMDT, tag="qT", bufs=1)
            kT_sb = pers.tile([P, HPC, B, T], MMDT, tag="kT", bufs=1)
            v_sb = pers.tile([P, B, T // P, HPC * D], MMDT, tag="v", bufs=1)
            wq_sb = pers.tile([P, HPC, CT, P], MMDT, tag="wq", bufs=1)
            wk_sb = pers.tile([P, HPC, CT, P], MMDT, tag="wk", bufs=1)
            wv_sb = pers.tile([P, CT, HPC * D], MMDT, tag="wv", bufs=1)
            for hl in range(HPC):
                nc.sync.dma_start(
                    out=wq_sb[:, hl],
                    in_=wq[:, hl * P : (hl + 1) * P].rearrange(
                        "(ct p) d -> p ct d", p=P
                    ),
                )
                nc.sync.dma_start(
                    out=wk_sb[:, hl],
                    in_=wk[:, hl * P : (hl + 1) * P].rearrange(
                        "(ct p) d -> p ct d", p=P
                    ),
                )
            nc.sync.dma_start(
                out=wv_sb, in_=wv.rearrange("(ct p) d -> p ct d", p=P)
            )

            with (
                tc.tile_pool(name="a1", bufs=2) as a1,
                tc.tile_pool(name="a1psum", bufs=2, space="PSUM") as pa1,
            ):
                for b in range(B):
                    for tci in range(T // CHW):
                        t0 = tci * CHW
                        xtc = a1.tile([P, CT, CHW], MMDT, tag="xtc", bufs=2)
                        nc.sync.dma_start(
                            out=xtc,
                            in_=x_tb[b, :, t0 : t0 + CHW].rearrange(
                                "(ct p) t -> p ct t", p=P
                            ),
                        )
                        # rstd for these tokens: colsum of x^2 via ones-matmul
                        rsp = pa1.tile([1, CHW], F32, tag="rsp", bufs=2)
                        for ct in range(CT):
                            sq = a1.tile([P, CHW], MMDT, tag="sq", bufs=3)
                            nc.vector.tensor_tensor(
                                out=sq, in0=xtc[:, ct, :], in1=xtc[:, ct, :],
                                op=ALU.mult,
                            )
                            nc.tensor.matmul(
                                rsp, ones1, sq, start=(ct == 0), stop=(ct == CT - 1)
                            )
                        rrow = a1.tile([1, CHW], F32, tag="rrow", bufs=2)
                        nc.scalar.activation(
                            rrow, rsp, AF.Sqrt, bias=eps_sb[0:1, :], scale=1.0 / C
                        )
                        rinv = a1.tile([1, CHW], F32, tag="rinv", bufs=2)
                        nc.vector.reciprocal(out=rinv, in_=rrow)
                        rinv_b = a1.tile([1, CHW], MMDT, tag="rinvb", bufs=2)
                        nc.vector.tensor_copy(out=rinv_b, in_=rinv)
                        rbc = a1.tile([P, CHW], MMDT, tag="rbc", bufs=2)
                        nc.gpsimd.partition_broadcast(rbc[:], rinv_b[:])
                        hTc = a1.tile([P, CT, CHW], MMDT, tag="hT", bufs=2)
                        for ct in range(CT):
                            nc.vector.tensor_tensor(
                                out=hTc[:, ct, :], in0=xtc[:, ct, :], in1=rbc,
                                op=ALU.mult,
                            )
                        # q^T / k^T with fused rope
                        for wsb, dst in ((wq_sb, qT_sb), (wk_sb, kT_sb)):
                            for hl in range(HPC):
                                pq = pa1.tile([P, CHW], F32, tag="pq", bufs=3)
                                for ct in range(CT):
                                    nc.tensor.matmul(
                                        pq,
                                        wsb[:, hl, ct, :],
                                        hTc[:, ct, :],
                                        start=(ct == 0),
                                        stop=(ct == CT - 1),
                                    )
                                x1 = pq[0:HD2, :]
                                x2 = pq[HD2:P, :]
                                cosw = rope_sb[0:HD2, t0 : t0 + CHW]
                                sinw = rope_sb[HD2:D, t0 : t0 + CHW]
                                tm1 = a1.tile([HD2, CHW], F32, tag="tm1", bufs=2)
                                tm2 = a1.tile([HD2, CHW], F32, tag="tm2", bufs=2)
                                dslc = dst[:, hl, b, t0 : t0 + CHW]
                                nc.vector.tensor_tensor(
                                    out=tm1, in0=x1, in1=cosw, op=ALU.mult
                                )
                                nc.vector.tensor_tensor(
                                    out=tm2, in0=x2, in1=sinw, op=ALU.mult
                                )
                                nc.vector.tensor_tensor(
                                    out=dslc[0:HD2], in0=tm1, in1=tm2,
                                    op=ALU.subtract,
                                )
                                nc.vector.tensor_tensor(
                                    out=tm1, in0=x1, in1=sinw, op=ALU.mult
                                )
                                nc.vector.tensor_tensor(
                                    out=tm2, in0=x2, in1=cosw, op=ALU.mult
                                )
                                nc.vector.tensor_tensor(
                                    out=dslc[HD2:P], in0=tm1, in1=tm2, op=ALU.add
                                )
                        # v rows
                        for rt in range(CHW // P):
                            pv = pa1.tile([P, HPC * D], F32, tag="pv", bufs=3)
                            for ct in range(CT):
                                nc.tensor.matmul(
                                    pv,
                                    hTc[:, ct, rt * P : (rt + 1) * P],
                                    wv_sb[:, ct, :],
                                    start=(ct == 0),
                                    stop=(ct == CT - 1),
                                )
                            nc.scalar.activation(
                                v_sb[:, b, tci * (CHW // P) + rt, :], pv, AF.Copy
                            )

            # ---- causal attention, all in SBUF ----
            with (
                tc.tile_pool(name="att", bufs=2) as att,
                tc.tile_pool(name="attpsum", bufs=2, space="PSUM") as pat,
            ):
                for hl in range(HPC):
                    a2a_in = a2a1_in if hl == 0 else a2a2_in
                    for b in range(B):
                        for qc in range(T // TQ):
                            nkb = 4 * qc + 4
                            l_ps = pat.tile([P, TQ], F32, tag="l", bufs=2)
                            o_ps = pat.tile([P, TQ], F32, tag="o", bufs=2)
                            es = []
                            for kb in range(nkb):
                                r = kb - 4 * qc
                                q0 = max(0, r * P)
                                st = pat.tile([P, TQ], F32, tag="st", bufs=4)
                                nc.tensor.matmul(
                                    st[:, q0:TQ],
                                    kT_sb[:, hl, b, kb * P : (kb + 1) * P],
                                    qT_sb[:, hl, b, qc * TQ + q0 : (qc + 1) * TQ],
                                    start=True,
                                    stop=True,
                                )
                                e = att.tile([P, TQ], MMDT, tag="e", bufs=18)
                                nc.scalar.activation(
                                    e[:, q0:TQ], st[:, q0:TQ], AF.Exp,
                                    scale=inv_sqrt_d,
                                )
                                if r >= 0:
                                    nc.vector.tensor_tensor(
                                        out=e[:, q0 : q0 + P],
                                        in0=e[:, q0 : q0 + P],
                                        in1=tri_sb,
                                        op=ALU.mult,
                                    )
                                es.append((e, q0))
                            for kb in range(nkb):
                                e, q0 = es[kb]
                                nc.tensor.matmul(
                                    l_ps[:, q0:TQ],
                                    ones128,
                                    e[:, q0:TQ],
                                    start=(kb == 0),
                                    stop=(kb == nkb - 1),
                                )
                                nc.tensor.matmul(
                                    o_ps[:, q0:TQ],
                                    v_sb[:, b, kb, hl * D : (hl + 1) * D],
                                    e[:, q0:TQ],
                                    start=(kb == 0),
                                    stop=(kb == nkb - 1),
                                )
                            l_inv = att.tile([P, TQ], F32, tag="linv", bufs=2)
                            nc.vector.reciprocal(out=l_inv, in_=l_ps)
                            oT = att.tile([P, TQ], MMDT, tag="oT", bufs=3)
                            nc.vector.tensor_tensor(
                                out=oT, in0=o_ps, in1=l_inv, op=ALU.mult
                            )
                            nc.sync.dma_start(
                                out=a2a_in[b * 4 + qc, :, :], in_=oT
                            )
                    if hl == 0:
                        nc.gpsimd.collective_compute(
                            "AllToAll",
                            ALU.bypass,
                            replica_groups=[[0, 1, 2, 3, 4, 5, 6, 7]],
                            ins=[a2a1_in.opt()],
                            outs=[a2a1_out.opt()],
                        )
                    else:
                        nc.gpsimd.collective_compute(
                            "AllToAll",
                            ALU.bypass,
                            replica_groups=[[0, 1, 2, 3, 4, 5, 6, 7]],
                            ins=[a2a2_in.opt()],
                            outs=[a2a2_out.opt()],
                        )

            pers_ctx.__exit__(None, None, None)

            # ================= Phase B (transposed) =================
            bres_ctx = tc.tile_pool(name="bres", bufs=1)
            bres = bres_ctx.__enter__()
            xmidT = bres.tile([P, CT, TQ], F32, tag="xmid", bufs=1)
            h2T = bres.tile([P, CT, TQ], MMDT, tag="h2T", bufs=1)

            with (
                tc.tile_pool(name="b1", bufs=2) as b1,
                tc.tile_pool(name="b1psum", bufs=2, space="PSUM") as pb1,
            ):
                xres_sb = b1.tile([P, CT, TQ], F32, tag="xres", bufs=1)
                nc.sync.dma_start(
                    out=xres_sb, in_=x_res.rearrange("(ct p) t -> p ct t", p=P)
                )
                lp0 = b1.tile([P, 8, TQ], MMDT, tag="lp0", bufs=1)
                nc.sync.dma_start(
                    out=lp0, in_=a2a1_out.rearrange("s p t -> p s t")
                )
                lp1 = b1.tile([P, 8, TQ], MMDT, tag="lp1", bufs=1)
                y0 = b1.tile([P, CT, TQ], F32, tag="y0", bufs=1)
                rsp2 = pb1.tile([1, TQ], F32, tag="rsp2", bufs=1)
                for half in range(2):
                    if half == 1:
                        # emitted after half-0's wpe loads: its A2A#2 wait must
                        # not block them in the sync-engine DMA FIFO
                        nc.sync.dma_start(
                            out=lp1, in_=a2a2_out.rearrange("s p t -> p s t")
                        )
                    lp = lp0 if half == 0 else lp1
                    for ctb in range(CT):
                        wpet = b1.tile([P, 8, P], MMDT, tag="wpet", bufs=4)
                        nc.sync.dma_start(
                            out=wpet,
                            in_=wpe[
                                half * 8 * P : (half * 8 + 8) * P,
                                ctb * P : (ctb + 1) * P,
                            ].rearrange("(blk p) c -> p blk c", p=P),
                        )
                        yps = pb1.tile([P, TQ], F32, tag="yps", bufs=2)
                        for s in range(8):
                            nc.tensor.matmul(
                                yps,
                                wpet[:, s, :],
                                lp[:, s, :],
                                start=(s == 0),
                                stop=(s == 7),
                            )
                        if half == 0:
                            nc.vector.tensor_copy(out=y0[:, ctb, :], in_=yps)
                        else:
                            t1 = b1.tile([P, TQ], F32, tag="t1", bufs=3)
                            nc.vector.tensor_tensor(
                                out=t1, in0=yps, in1=y0[:, ctb, :], op=ALU.add
                            )
                            nc.vector.tensor_tensor(
                                out=xmidT[:, ctb, :], in0=t1,
                                in1=xres_sb[:, ctb, :], op=ALU.add,
                            )
                            sq2 = b1.tile([P, TQ], MMDT, tag="sq2", bufs=3)
                            nc.vector.tensor_tensor(
                                out=sq2, in0=xmidT[:, ctb, :],
                                in1=xmidT[:, ctb, :], op=ALU.mult,
                            )
                            nc.tensor.matmul(
                                rsp2, ones1, sq2,
                                start=(ctb == 0), stop=(ctb == CT - 1),
                            )
                rrow2 = b1.tile([1, TQ], F32, tag="rrow2", bufs=1)
                nc.scalar.activation(
                    rrow2, rsp2, AF.Sqrt, bias=eps_sb[0:1, :], scale=1.0 / C
                )
                rinv2 = b1.tile([1, TQ], F32, tag="rinv2", bufs=1)
                nc.vector.reciprocal(out=rinv2, in_=rrow2)
                rbc2 = b1.tile([P, TQ], F32, tag="rbc2", bufs=1)
                nc.gpsimd.partition_broadcast(rbc2[:], rinv2[:])
                for ctb in range(CT):
                    nc.vector.tensor_tensor(
                        out=h2T[:, ctb, :], in0=xmidT[:, ctb, :], in1=rbc2,
                        op=ALU.mult,
                    )

            # ---- SwiGLU MLP ----
            with tc.tile_pool(name="mlp", bufs=2) as mlp:
                uT = mlp.tile([P, HID_T, TQ], MMDT, tag="uT", bufs=1)
                with tc.tile_pool(name="mlpg", bufs=2, space="PSUM") as pg:
                    for ht in range(HID_T):
                        w1_sb = mlp.tile([P, CT, P], MMDT, tag="w1s", bufs=3)
                        nc.sync.dma_start(
                            out=w1_sb,
                            in_=w1t[ht].rearrange("(ct p) d -> p ct d", p=P),
                        )
                        w2_sb = mlp.tile([P, CT, P], MMDT, tag="w2s", bufs=3)
                        nc.sync.dma_start(
                            out=w2_sb,
                            in_=w2t[ht].rearrange("(ct p) d -> p ct d", p=P),
                        )
                        g1 = pg.tile([P, TQ], F32, tag="g1", bufs=2)
                        g2 = pg.tile([P, TQ], F32, tag="g2", bufs=2)
                        for ct in range(CT):
                            nc.tensor.matmul(
                                g1, w1_sb[:, ct, :], h2T[:, ct, :],
                                start=(ct == 0), stop=(ct == CT - 1),
                            )
                        for ct in range(CT):
                            nc.tensor.matmul(
                                g2, w2_sb[:, ct, :], h2T[:, ct, :],
                                start=(ct == 0), stop=(ct == CT - 1),
                            )
                        sil = mlp.tile([P, TQ], F32, tag="sil", bufs=3)
                        nc.scalar.activation(sil, g1, AF.Silu)
                        nc.vector.tensor_tensor(
                            out=uT[:, ht, :], in0=g2, in1=sil, op=ALU.mult
                        )
                with tc.tile_pool(name="mlpy", bufs=1, space="PSUM") as py:
                    for g in range(2):
                        y3ps = [
                            py.tile(
                                [P, TQ], F32, tag=f"y3_{ci}", bufs=1,
                                name=f"y3_{ci}",
                            )
                            for ci in range(8)
                        ]
                        for ht in range(HID_T):
                            w3_sb = mlp.tile([P, 8 * P], MMDT, tag="w3s", bufs=3)
                            nc.sync.dma_start(
                                out=w3_sb,
                                in_=w3[
                                    ht * P : (ht + 1) * P,
                                    g * 8 * P : (g + 1) * 8 * P,
                                ],
                            )
                            for ci in range(8):
                                nc.tensor.matmul(
                                    y3ps[ci],
                                    w3_sb[:, ci * P : (ci + 1) * P],
                                    uT[:, ht, :],
                                    start=(ht == 0),
                                    stop=(ht == HID_T - 1),
                                )
                        for ci in range(8):
                            ctb = g * 8 + ci
                            of = mlp.tile([P, TQ], F32, tag="of", bufs=3)
                            nc.vector.tensor_tensor(
                                out=of, in0=y3ps[ci], in1=xmidT[:, ctb, :],
                                op=ALU.add,
                            )
                            nc.sync.dma_start(
                                out=out_t[ctb * P : (ctb + 1) * P, :], in_=of
                            )

            bres_ctx.__exit__(None, None, None)

    nc.compile()
    return nc


_NC_CACHE = None


def _get_nc():
    global _NC_CACHE
    if _NC_CACHE is None:
        _NC_CACHE = _build()
    return _NC_CACHE


def _host_inputs(x, w_norm1, w_qkv, w_proj, w_norm2, w1, w2, w3):
    x = np.asarray(x, dtype=np.float32)
    w_qkv = np.asarray(w_qkv, dtype=np.float32)
    w_proj = np.asarray(w_proj, dtype=np.float32)
    w_norm1 = np.asarray(w_norm1, dtype=np.float32)
    w_norm2 = np.asarray(w_norm2, dtype=np.float32)
    w1 = np.asarray(w1, dtype=np.float32)
    w2 = np.asarray(w2, dtype=np.float32)
    w3 = np.asarray(w3, dtype=np.float32)

    inv_freq = 1.0 / (ROPE_BASE ** (np.arange(HD2, dtype=np.float32) / HD2))
    pos = np.arange(T, dtype=np.float32)
    freqs = pos[:, None] * inv_freq[None, :]
    rope_tab = np.ascontiguousarray(
        np.concatenate([np.cos(freqs).T, np.sin(freqs).T], axis=0).astype(np.float32)
    )

    ql = np.arange(P)[None, :]
    kv = np.arange(P)[:, None]
    tri = (ql >= kv).astype(NP_MMDT)

    # fold w_norm into weight rows (h @ W == (x*rstd) @ (diag(wn) W))
    w_qkv_n = w_qkv * w_norm1[:, None]
    w1_n = w1 * w_norm2[:, None]
    w2_n = w2 * w_norm2[:, None]

    # shared across cores
    x_tb = np.ascontiguousarray(x.transpose(0, 2, 1)).astype(NP_MMDT)
    x_t32 = np.ascontiguousarray(x.transpose(0, 2, 1))
    w1t = np.ascontiguousarray(
        w1_n.reshape(C, HID_T, P).transpose(1, 0, 2)
    ).astype(NP_MMDT)
    w2t = np.ascontiguousarray(
        w2_n.reshape(C, HID_T, P).transpose(1, 0, 2)
    ).astype(NP_MMDT)
    w3_b = w3.astype(NP_MMDT)
    # wpe row block (k*8+s) <- w_proj rows of head 2s+k
    perm = np.empty(16, dtype=np.int64)
    for k in range(2):
        for s in range(8):
            perm[k * 8 + s] = 2 * s + k
    wpe = np.ascontiguousarray(
        w_proj.reshape(16, P, C)[perm].reshape(C, C)
    ).astype(NP_MMDT)

    in_maps = []
    for j in range(8):
        b, hg = j // 4, j % 4
        col0 = 2 * j * D
        in_maps.append(
            {
                "x_tb": x_tb,
                "x_res": np.ascontiguousarray(
                    x_t32[b, :, hg * TQ : (hg + 1) * TQ]
                ),
                "wq": np.ascontiguousarray(
                    w_qkv_n[:, col0 : col0 + HPC * D]
                ).astype(NP_MMDT),
                "wk": np.ascontiguousarray(
                    w_qkv_n[:, C + col0 : C + col0 + HPC * D]
                ).astype(NP_MMDT),
                "wv": np.ascontiguousarray(
                    w_qkv_n[:, 2 * C + col0 : 2 * C + col0 + HPC * D]
                ).astype(NP_MMDT),
                "wpe": wpe,
                "w1t": w1t,
                "w2t": w2t,
                "w3": w3_b,
                "rope_t": rope_tab,
                "tri": tri,
            }
        )
    return in_maps


def kernel(x, w_norm1, w_qkv, w_proj, w_norm2, w1, w2, w3, _trace=False, _tmpdir=None):
    nc = _get_nc()
    in_maps = _host_inputs(x, w_norm1, w_qkv, w_proj, w_norm2, w1, w2, w3)
    kwargs = {}
    if _trace:
        kwargs = {"trace": True, "tmpdir": _tmpdir}
    res = bass_utils.run_bass_kernel_spmd(
        nc, in_maps, core_ids=list(range(8)), **kwargs
    )
    out = np.empty((B, T, C), dtype=np.float32)
    for j in range(8):
        out[j // 4, (j % 4) * TQ : (j % 4 + 1) * TQ, :] = res.results[j]["out_t"].T
    kernel._last_exec_time_ns = res.exec_time_ns
    return out


# revision 17
# speedup vs baseline: 1.2924x; 1.0508x over previous
"""Dense transformer block (rmsnorm+causal attention+rope / rmsnorm+SwiGLU) on 8 TRN2 cores.

Sharding (v2):
  core j owns head pair {2j, 2j+1} for BOTH batches (attention head-parallel),
  and owns output row-chunk (b=j//4, rows (j%4)*512..) for phase B (row-parallel).

  Phase A: rmsnorm1 (rstd via all-ones matmul on x^T) -> h^T chunks -> QKV for the
  2 heads x 2 batches -> rope -> causal attention entirely in SBUF.
  Two 8-core AllToAlls reshard head->row: A2A#1 carries head 2j (fires at 50% of
  attention), A2A#2 carries head 2j+1 (fires at the end). Each slice r=(b*4+qc)
  is [128, 512] -> every byte is useful (no cross-batch padding).

  Phase B runs fully transposed (c on partitions, tokens on free dim):
  proj y^T accumulated per 128-col block of C over 16 received head blocks,
  residual from x^T slice, rmsnorm2 via all-ones matmul, SwiGLU with u^T kept
  for all 44 hidden tiles, w3 pass accumulates y3^T per c-block in PSUM.
  Output is written transposed [C, 512]; the host transposes back.

Matmul operands bf16 (weights pre-cast, w_norm folded); stats/PSUM fp32.
"""

import numpy as np
import ml_dtypes

import concourse.bass as bass
import concourse.mybir as mybir
import concourse.tile as tile
from concourse import bacc
from concourse import bass_utils
from concourse.masks import make_identity

AF = mybir.ActivationFunctionType
ALU = mybir.AluOpType
F32 = mybir.dt.float32
BF16 = mybir.dt.bfloat16
MMDT = BF16
NP_MMDT = ml_dtypes.bfloat16

P = 128
T = 2048
C = 2048
D = 128
HD2 = D // 2
H = 16
HPC = 2          # heads per core
B = 2
HID = 5632
HID_T = HID // P  # 44
TQ = 512
CT = C // P      # 16
CHW = 512        # token chunk width in phase A
EPS = 1e-6
ROPE_BASE = 10000.0


def _build():
    nc = bacc.Bacc(None, target_bir_lowering=False, num_devices=8)

    # ---- kernel I/O ----
    x_tb = nc.dram_tensor("x_tb", [B, C, T], MMDT, kind="ExternalInput")
    x_res = nc.dram_tensor("x_res", [C, TQ], F32, kind="ExternalInput")
    wq = nc.dram_tensor("wq", [C, HPC * D], MMDT, kind="ExternalInput")
    wk = nc.dram_tensor("wk", [C, HPC * D], MMDT, kind="ExternalInput")
    wv = nc.dram_tensor("wv", [C, HPC * D], MMDT, kind="ExternalInput")
    wpe = nc.dram_tensor("wpe", [C, C], MMDT, kind="ExternalInput")
    w1t = nc.dram_tensor("w1t", [HID_T, C, P], MMDT, kind="ExternalInput")
    w2t = nc.dram_tensor("w2t", [HID_T, C, P], MMDT, kind="ExternalInput")
    w3 = nc.dram_tensor("w3", [HID, C], MMDT, kind="ExternalInput")
    rope_t = nc.dram_tensor("rope_t", [D, T], F32, kind="ExternalInput")
    rope_sw = nc.dram_tensor("rope_sw", [D, T], F32, kind="ExternalInput")
    tri = nc.dram_tensor("tri", [P, P], MMDT, kind="ExternalInput")
    out_t = nc.dram_tensor("out_t", [C, TQ], F32, kind="ExternalOutput")

    inv_sqrt_d = 1.0 / float(np.sqrt(D))

    with tile.TileContext(nc) as tc:
        with (
            tc.tile_pool(name="const", bufs=1) as const,
            tc.tile_pool(name="dram", bufs=1, space="DRAM") as dram,
        ):
            # ---- constants (no DMA yet: first x chunk loads first) ----
            ones_f = const.tile([P, P], F32)
            nc.vector.memset(ones_f, 1.0)
            ones128 = const.tile([P, P], MMDT)
            nc.vector.tensor_copy(out=ones128, in_=ones_f)
            ones1 = const.tile([P, 1], MMDT)
            nc.vector.tensor_copy(out=ones1, in_=ones_f[:, 0:1])
            eps_sb = const.tile([P, 1], F32)
            nc.vector.memset(eps_sb, EPS)
            ident_f = const.tile([P, P], F32)
            make_identity(nc, ident_f)
            rope_sb = const.tile([D, T], F32)
            rope_sw_sb = const.tile([D, T], F32)
            tri_sb = const.tile([P, P], MMDT)

            # ---- DRAM scratch for collectives ----
            a2a1_in = dram.tile([8, P, TQ], MMDT)
            a2a1_out = dram.tile([8, P, TQ], MMDT)
            a2a2_in = dram.tile([8, P, TQ], MMDT)
            a2a2_out = dram.tile([8, P, TQ], MMDT)

            # ================= Phase A: QKV + attention =================
            pers_ctx = tc.tile_pool(name="pers", bufs=1)
            pers = pers_ctx.__enter__()
            qT_sb = pers.tile([P, HPC, B, T], MMDT, tag="qT", bufs=1)
            kT_sb = pers.tile([P, HPC, B, T], MMDT, tag="kT", bufs=1)
            v_sb = pers.tile([P, B, T // P, HPC * D], MMDT, tag="v", bufs=1)
            wq_sb = pers.tile([P, HPC, CT, P], MMDT, tag="wq", bufs=1)
            wk_sb = pers.tile([P, HPC, CT, P], MMDT, tag="wk", bufs=1)
            wv_sb = pers.tile([P, CT, HPC * D], MMDT, tag="wv", bufs=1)

            with (
                tc.tile_pool(name="a1", bufs=2) as a1,
                tc.tile_pool(name="a1psum", bufs=2, space="PSUM") as pa1,
            ):
                # first x chunk loads before the weights so PE starts early
                xtc0 = a1.tile([P, CT, CHW], MMDT, tag="xtc", bufs=2, name="xtc0")
                nc.sync.dma_start(
                    out=xtc0,
                    in_=x_tb[0, :, 0:CHW].rearrange("(ct p) t -> p ct t", p=P),
                )
                for hl in range(HPC):
                    nc.sync.dma_start(
                        out=wq_sb[:, hl],
                        in_=wq[:, hl * P : (hl + 1) * P].rearrange(
                            "(ct p) d -> p ct d", p=P
                        ),
                    )
                    nc.sync.dma_start(
                        out=wk_sb[:, hl],
                        in_=wk[:, hl * P : (hl + 1) * P].rearrange(
                            "(ct p) d -> p ct d", p=P
                        ),
                    )
                nc.sync.dma_start(out=rope_sb, in_=rope_t[:, :])
                nc.sync.dma_start(out=rope_sw_sb, in_=rope_sw[:, :])
                nc.sync.dma_start(
                    out=wv_sb, in_=wv.rearrange("(ct p) d -> p ct d", p=P)
                )
                nc.sync.dma_start(out=tri_sb, in_=tri[:, :])

                for b in range(B):
                    for tci in range(T // CHW):
                        t0 = tci * CHW
                        if b == 0 and tci == 0:
                            xtc = xtc0
                        else:
                            xtc = a1.tile([P, CT, CHW], MMDT, tag="xtc", bufs=2)
                            nc.sync.dma_start(
                                out=xtc,
                                in_=x_tb[b, :, t0 : t0 + CHW].rearrange(
                                    "(ct p) t -> p ct t", p=P
                                ),
                            )
                        # rstd for these tokens: colsum of x^2 via ones-matmul.
                        # rstd is a per-token scale, so it commutes through the
                        # QKV contraction: fold into rope tables (q/k) and into
                        # a per-partition scale on v. q/k/v consume raw x.
                        rsp = pa1.tile([1, CHW], F32, tag="rsp", bufs=2)
                        for ct in range(CT):
                            sq = a1.tile([P, CHW], MMDT, tag="sq", bufs=3)
                            nc.vector.tensor_tensor(
                                out=sq, in0=xtc[:, ct, :], in1=xtc[:, ct, :],
                                op=ALU.mult,
                            )
                            nc.tensor.matmul(
                                rsp, ones1, sq, start=(ct == 0), stop=(ct == CT - 1)
                            )
                        rrow = a1.tile([1, CHW], F32, tag="rrow", bufs=2)
                        nc.scalar.activation(
                            rrow, rsp, AF.Sqrt, bias=eps_sb[0:1, :], scale=1.0 / C
                        )
                        rinv = a1.tile([1, CHW], F32, tag="rinv", bufs=2)
                        nc.vector.reciprocal(out=rinv, in_=rrow)
                        rbc = a1.tile([P, CHW], F32, tag="rbc", bufs=2)
                        nc.gpsimd.partition_broadcast(rbc[:], rinv[:])
                        # rcs rows 0:64 = cos*r, 64:128 = sin*r; rcs_sw swapped
                        rcs = a1.tile([D, CHW], F32, tag="rcs", bufs=2)
                        rcs_sw = a1.tile([D, CHW], F32, tag="rcssw", bufs=2)
                        nc.vector.tensor_tensor(
                            out=rcs, in0=rope_sb[:, t0 : t0 + CHW], in1=rbc,
                            op=ALU.mult,
                        )
                        nc.vector.tensor_tensor(
                            out=rcs_sw, in0=rope_sw_sb[:, t0 : t0 + CHW], in1=rbc,
                            op=ALU.mult,
                        )
                        # q^T / k^T from raw x with rstd-folded rope
                        for wsb, dst in ((wq_sb, qT_sb), (wk_sb, kT_sb)):
                            for hl in range(HPC):
                                pq = pa1.tile([P, CHW], F32, tag="pq", bufs=2)
                                for ct in range(CT):
                                    nc.tensor.matmul(
                                        pq,
                                        wsb[:, hl, ct, :],
                                        xtc[:, ct, :],
                                        start=(ct == 0),
                                        stop=(ct == CT - 1),
                                    )
                                x1 = pq[0:HD2, :]
                                x2 = pq[HD2:P, :]
                                tm1 = a1.tile([HD2, CHW], F32, tag="tm1", bufs=2)
                                tm2 = a1.tile([HD2, CHW], F32, tag="tm2", bufs=2)
                                dslc = dst[:, hl, b, t0 : t0 + CHW]
                                nc.vector.tensor_tensor(
                                    out=tm1, in0=x1, in1=rcs[0:HD2, :], op=ALU.mult
                                )
                                nc.vector.tensor_tensor(
                                    out=tm2, in0=x2, in1=rcs[HD2:D, :], op=ALU.mult
                                )
                                nc.vector.tensor_tensor(
                                    out=dslc[0:HD2], in0=tm1, in1=tm2,
                                    op=ALU.subtract,
                                )
                                nc.vector.tensor_tensor(
                                    out=tm1, in0=x1, in1=rcs_sw[0:HD2, :],
                                    op=ALU.mult,
                                )
                                nc.vector.tensor_tensor(
                                    out=tm2, in0=x2, in1=rcs_sw[HD2:D, :],
                                    op=ALU.mult,
                                )
                                nc.vector.tensor_tensor(
                                    out=dslc[HD2:P], in0=tm1, in1=tm2, op=ALU.add
                                )
                        # rstd as per-partition column for the v rows
                        rcols = []
                        for rt in range(CHW // P):
                            rc_ps = pa1.tile([P, 1], F32, tag="rcps", bufs=2)
                            nc.tensor.matmul(
                                rc_ps,
                                rinv[0:1, rt * P : (rt + 1) * P],
                                ones_f[0:1, 0:1],
                                start=True,
                                stop=True,
                            )
                            rcol = a1.tile([P, 1], F32, tag="rcol", bufs=4)
                            nc.scalar.activation(rcol, rc_ps, AF.Copy)
                            rcols.append(rcol)
                        # v rows from raw x, scaled by rstd on eviction
                        for rt in range(CHW // P):
                            pv = pa1.tile([P, HPC * D], F32, tag="pv", bufs=2)
                            for ct in range(CT):
                                nc.tensor.matmul(
                                    pv,
                                    xtc[:, ct, rt * P : (rt + 1) * P],
                                    wv_sb[:, ct, :],
                                    start=(ct == 0),
                                    stop=(ct == CT - 1),
                                )
                            nc.vector.tensor_scalar(
                                out=v_sb[:, b, tci * (CHW // P) + rt, :],
                                in0=pv,
                                scalar1=rcols[rt],
                                scalar2=None,
                                op0=ALU.mult,
                            )

            # ---- causal attention, all in SBUF ----
            with (
                tc.tile_pool(name="att", bufs=2) as att,
                tc.tile_pool(name="attpsum", bufs=2, space="PSUM") as pat,
            ):
                for hl in range(HPC):
                    a2a_in = a2a1_in if hl == 0 else a2a2_in
                    for b in range(B):
                        for qc in range(T // TQ):
                            nkb = 4 * qc + 4
                            l_ps = pat.tile([P, TQ], F32, tag="l", bufs=2)
                            o_ps = pat.tile([P, TQ], F32, tag="o", bufs=2)
                            es = []
                            for kb in range(nkb):
                                r = kb - 4 * qc
                                q0 = max(0, r * P)
                                st = pat.tile([P, TQ], F32, tag="st", bufs=4)
                                nc.tensor.matmul(
                                    st[:, q0:TQ],
                                    kT_sb[:, hl, b, kb * P : (kb + 1) * P],
                                    qT_sb[:, hl, b, qc * TQ + q0 : (qc + 1) * TQ],
                                    start=True,
                                    stop=True,
                                )
                                e = att.tile([P, TQ], MMDT, tag="e", bufs=18)
                                nc.scalar.activation(
                                    e[:, q0:TQ], st[:, q0:TQ], AF.Exp,
                                    scale=inv_sqrt_d,
                                )
                                if r >= 0:
                                    nc.vector.tensor_tensor(
                                        out=e[:, q0 : q0 + P],
                                        in0=e[:, q0 : q0 + P],
                                        in1=tri_sb,
                                        op=ALU.mult,
                                    )
                                es.append((e, q0))
                            for kb in range(nkb):
                                e, q0 = es[kb]
                                nc.tensor.matmul(
                                    l_ps[:, q0:TQ],
                                    ones128,
                                    e[:, q0:TQ],
                                    start=(kb == 0),
                                    stop=(kb == nkb - 1),
                                )
                                nc.tensor.matmul(
                                    o_ps[:, q0:TQ],
                                    v_sb[:, b, kb, hl * D : (hl + 1) * D],
                                    e[:, q0:TQ],
                                    start=(kb == 0),
                                    stop=(kb == nkb - 1),
                                )
                            l_inv = att.tile([P, TQ], F32, tag="linv", bufs=2)
                            nc.vector.reciprocal(out=l_inv, in_=l_ps)
                            oT = att.tile([P, TQ], MMDT, tag="oT", bufs=3)
                            nc.vector.tensor_tensor(
                                out=oT, in0=o_ps, in1=l_inv, op=ALU.mult
                            )
                            nc.sync.dma_start(
                                out=a2a_in[b * 4 + qc, :, :], in_=oT
                            )
                    if hl == 0:
                        nc.gpsimd.collective_compute(
                            "AllToAll",
                            ALU.bypass,
                            replica_groups=[[0, 1, 2, 3, 4, 5, 6, 7]],
                            ins=[a2a1_in.opt()],
                            outs=[a2a1_out.opt()],
                        )
                    else:
                        nc.gpsimd.collective_compute(
                            "AllToAll",
                            ALU.bypass,
                            replica_groups=[[0, 1, 2, 3, 4, 5, 6, 7]],
                            ins=[a2a2_in.opt()],
                            outs=[a2a2_out.opt()],
                        )

            pers_ctx.__exit__(None, None, None)

            # ================= Phase B (transposed) =================
            bres_ctx = tc.tile_pool(name="bres", bufs=1)
            bres = bres_ctx.__enter__()
            xmidT = bres.tile([P, CT, TQ], F32, tag="xmid", bufs=1)
            xmid_bf = bres.tile([P, CT, TQ], MMDT, tag="xmidbf", bufs=1)
            h2T = bres.tile([P, CT, TQ], MMDT, tag="h2T", bufs=1)
            mlp_ctx = tc.tile_pool(name="mlp", bufs=2)
            mlp = mlp_ctx.__enter__()
            # prefetch first MLP weight tiles (DMA runs during attention/proj)
            w12_pref = []
            for ht in range(2):
                w1_p = mlp.tile([P, CT, P], MMDT, tag="w1s", bufs=3, name="w1_p")
                nc.sync.dma_start(
                    out=w1_p, in_=w1t[ht].rearrange("(ct p) d -> p ct d", p=P)
                )
                w2_p = mlp.tile([P, CT, P], MMDT, tag="w2s", bufs=3, name="w2_p")
                nc.sync.dma_start(
                    out=w2_p, in_=w2t[ht].rearrange("(ct p) d -> p ct d", p=P)
                )
                w12_pref.append((w1_p, w2_p))

            with (
                tc.tile_pool(name="b1", bufs=2) as b1,
                tc.tile_pool(name="b1psum", bufs=2, space="PSUM") as pb1,
            ):
                lp0 = b1.tile([P, 8, TQ], MMDT, tag="lp0", bufs=1)
                nc.sync.dma_start(
                    out=lp0, in_=a2a1_out.rearrange("s p t -> p s t")
                )
                lp1 = b1.tile([P, 8, TQ], MMDT, tag="lp1", bufs=1)
                rsp2 = pb1.tile([1, TQ], F32, tag="rsp2", bufs=1)
                for half in range(2):
                    if half == 1:
                        # emitted after half-0's wpe loads: its A2A#2 wait must
                        # not block them in the sync-engine DMA FIFO
                        nc.sync.dma_start(
                            out=lp1, in_=a2a2_out.rearrange("s p t -> p s t")
                        )
                    lp = lp0 if half == 0 else lp1
                    for ctb in range(CT):
                        wpet = b1.tile([P, 8, P], MMDT, tag="wpet", bufs=3)
                        nc.sync.dma_start(
                            out=wpet,
                            in_=wpe[
                                half * 8 * P : (half * 8 + 8) * P,
                                ctb * P : (ctb + 1) * P,
                            ].rearrange("(blk p) c -> p blk c", p=P),
                        )
                        if half == 0:
                            xres_t = b1.tile([P, TQ], F32, tag="xrest", bufs=3)
                            nc.sync.dma_start(
                                out=xres_t, in_=x_res[ctb * P : (ctb + 1) * P, :]
                            )
                        yps = pb1.tile([P, TQ], F32, tag="yps", bufs=2)
                        for s in range(8):
                            nc.tensor.matmul(
                                yps,
                                wpet[:, s, :],
                                lp[:, s, :],
                                start=(s == 0),
                                stop=(s == 7),
                            )
                        if half == 0:
                            nc.vector.tensor_tensor(
                                out=xmidT[:, ctb, :], in0=yps, in1=xres_t,
                                op=ALU.add,
                            )
                        else:
                            nc.vector.tensor_tensor(
                                out=xmidT[:, ctb, :], in0=yps,
                                in1=xmidT[:, ctb, :], op=ALU.add,
                            )
                            nc.vector.tensor_copy(
                                out=xmid_bf[:, ctb, :], in_=xmidT[:, ctb, :]
                            )
                            sq2 = b1.tile([P, TQ], MMDT, tag="sq2", bufs=3)
                            nc.vector.tensor_tensor(
                                out=sq2, in0=xmidT[:, ctb, :],
                                in1=xmidT[:, ctb, :], op=ALU.mult,
                            )
                            nc.tensor.matmul(
                                rsp2, ones1, sq2,
                                start=(ctb == 0), stop=(ctb == CT - 1),
                            )
                rrow2 = b1.tile([1, TQ], F32, tag="rrow2", bufs=1)
                nc.scalar.activation(
                    rrow2, rsp2, AF.Sqrt, bias=eps_sb[0:1, :], scale=1.0 / C
                )
                rinv2 = b1.tile([1, TQ], F32, tag="rinv2", bufs=1)
                nc.vector.reciprocal(out=rinv2, in_=rrow2)
                rinv2b = b1.tile([1, TQ], MMDT, tag="rinv2b", bufs=1)
                nc.vector.tensor_copy(out=rinv2b, in_=rinv2)
                rbc2 = b1.tile([P, TQ], MMDT, tag="rbc2", bufs=1)
                nc.gpsimd.partition_broadcast(rbc2[:], rinv2b[:])
                for ctb in range(CT):
                    nc.vector.tensor_tensor(
                        out=h2T[:, ctb, :], in0=xmid_bf[:, ctb, :], in1=rbc2,
                        op=ALU.mult,
                    )

            # ---- SwiGLU MLP ----
            if True:
                uT = mlp.tile([P, HID_T, TQ], MMDT, tag="uT", bufs=1)
                with tc.tile_pool(name="mlpg", bufs=2, space="PSUM") as pg:
                    for ht in range(HID_T):
                        if ht < len(w12_pref):
                            w1_sb, w2_sb = w12_pref[ht]
                        else:
                            w1_sb = mlp.tile([P, CT, P], MMDT, tag="w1s", bufs=3)
                            nc.sync.dma_start(
                                out=w1_sb,
                                in_=w1t[ht].rearrange("(ct p) d -> p ct d", p=P),
                            )
                            w2_sb = mlp.tile([P, CT, P], MMDT, tag="w2s", bufs=3)
                            nc.sync.dma_start(
                                out=w2_sb,
                                in_=w2t[ht].rearrange("(ct p) d -> p ct d", p=P),
                            )
                        g1 = pg.tile([P, TQ], F32, tag="g1", bufs=2)
                        g2 = pg.tile([P, TQ], F32, tag="g2", bufs=2)
                        for ct in range(CT):
                            nc.tensor.matmul(
                                g1, w1_sb[:, ct, :], h2T[:, ct, :],
                                start=(ct == 0), stop=(ct == CT - 1),
                            )
                        for ct in range(CT):
                            nc.tensor.matmul(
                                g2, w2_sb[:, ct, :], h2T[:, ct, :],
                                start=(ct == 0), stop=(ct == CT - 1),
                            )
                        sil = mlp.tile([P, TQ], F32, tag="sil", bufs=3)
                        nc.scalar.activation(sil, g1, AF.Silu)
                        nc.vector.tensor_tensor(
                            out=uT[:, ht, :], in0=g2, in1=sil, op=ALU.mult
                        )
                with tc.tile_pool(name="mlpy", bufs=1, space="PSUM") as py:
                    for g in range(2):
                        y3ps = [
                            py.tile(
                                [P, TQ], F32, tag=f"y3_{ci}", bufs=1,
                                name=f"y3_{ci}",
                            )
                            for ci in range(8)
                        ]
                        for ht in range(HID_T):
                            w3_sb = mlp.tile([P, 8 * P], MMDT, tag="w3s", bufs=3)
                            nc.sync.dma_start(
                                out=w3_sb,
                                in_=w3[
                                    ht * P : (ht + 1) * P,
                                    g * 8 * P : (g + 1) * 8 * P,
                                ],
                            )
                            for ci in range(8):
                                nc.tensor.matmul(
                                    y3ps[ci],
                                    w3_sb[:, ci * P : (ci + 1) * P],
                                    uT[:, ht, :],
                                    start=(ht == 0),
                                    stop=(ht == HID_T - 1),
                                )
                        for ci in range(8):
                            ctb = g * 8 + ci
                            of = mlp.tile([P, TQ], F32, tag="of", bufs=3)
                            nc.vector.tensor_tensor(
                                out=of, in0=y3ps[ci], in1=xmidT[:, ctb, :],
                                op=ALU.add,
                            )
                            nc.sync.dma_start(
                                out=out_t[ctb * P : (ctb + 1) * P, :], in_=of
                            )

            mlp_ctx.__exit__(None, None, None)
            bres_ctx.__exit__(None, None, None)

    nc.compile()
    return nc


_NC_CACHE = None


def _get_nc():
    global _NC_CACHE
    if _NC_CACHE is None:
        _NC_CACHE = _build()
    return _NC_CACHE


def _host_inputs(x, w_norm1, w_qkv, w_proj, w_norm2, w1, w2, w3):
    x = np.asarray(x, dtype=np.float32)
    w_qkv = np.asarray(w_qkv, dtype=np.float32)
    w_proj = np.asarray(w_proj, dtype=np.float32)
    w_norm1 = np.asarray(w_norm1, dtype=np.float32)
    w_norm2 = np.asarray(w_norm2, dtype=np.float32)
    w1 = np.asarray(w1, dtype=np.float32)
    w2 = np.asarray(w2, dtype=np.float32)
    w3 = np.asarray(w3, dtype=np.float32)

    inv_freq = 1.0 / (ROPE_BASE ** (np.arange(HD2, dtype=np.float32) / HD2))
    pos = np.arange(T, dtype=np.float32)
    freqs = pos[:, None] * inv_freq[None, :]
    rope_tab = np.ascontiguousarray(
        np.concatenate([np.cos(freqs).T, np.sin(freqs).T], axis=0).astype(np.float32)
    )
    rope_tab_sw = np.ascontiguousarray(
        np.concatenate([np.sin(freqs).T, np.cos(freqs).T], axis=0).astype(np.float32)
    )

    ql = np.arange(P)[None, :]
    kv = np.arange(P)[:, None]
    tri = (ql >= kv).astype(NP_MMDT)

    # fold w_norm into weight rows (h @ W == (x*rstd) @ (diag(wn) W))
    w_qkv_n = w_qkv * w_norm1[:, None]
    w1_n = w1 * w_norm2[:, None]
    w2_n = w2 * w_norm2[:, None]

    # shared across cores
    x_tb = np.ascontiguousarray(x.transpose(0, 2, 1)).astype(NP_MMDT)
    x_t32 = np.ascontiguousarray(x.transpose(0, 2, 1))
    w1t = np.ascontiguousarray(
        w1_n.reshape(C, HID_T, P).transpose(1, 0, 2)
    ).astype(NP_MMDT)
    w2t = np.ascontiguousarray(
        w2_n.reshape(C, HID_T, P).transpose(1, 0, 2)
    ).astype(NP_MMDT)
    w3_b = w3.astype(NP_MMDT)
    # wpe row block (k*8+s) <- w_proj rows of head 2s+k
    perm = np.empty(16, dtype=np.int64)
    for k in range(2):
        for s in range(8):
            perm[k * 8 + s] = 2 * s + k
    wpe = np.ascontiguousarray(
        w_proj.reshape(16, P, C)[perm].reshape(C, C)
    ).astype(NP_MMDT)

    in_maps = []
    for j in range(8):
        b, hg = j // 4, j % 4
        col0 = 2 * j * D
        in_maps.append(
            {
                "x_tb": x_tb,
                "x_res": np.ascontiguousarray(
                    x_t32[b, :, hg * TQ : (hg + 1) * TQ]
                ),
                "wq": np.ascontiguousarray(
                    w_qkv_n[:, col0 : col0 + HPC * D]
                ).astype(NP_MMDT),
                "wk": np.ascontiguousarray(
                    w_qkv_n[:, C + col0 : C + col0 + HPC * D]
                ).astype(NP_MMDT),
                "wv": np.ascontiguousarray(
                    w_qkv_n[:, 2 * C + col0 : 2 * C + col0 + HPC * D]
                ).astype(NP_MMDT),
                "wpe": wpe,
                "w1t": w1t,
                "w2t": w2t,
                "w3": w3_b,
                "rope_t": rope_tab,
                "rope_sw": rope_tab_sw,
                "tri": tri,
            }
        )
    return in_maps


def kernel(x, w_norm1, w_qkv, w_proj, w_norm2, w1, w2, w3, _trace=False, _tmpdir=None):
    nc = _get_nc()
    in_maps = _host_inputs(x, w_norm1, w_qkv, w_proj, w_norm2, w1, w2, w3)
    kwargs = {}
    if _trace:
        kwargs = {"trace": True, "tmpdir": _tmpdir}
    res = bass_utils.run_bass_kernel_spmd(
        nc, in_maps, core_ids=list(range(8)), **kwargs
    )
    out = np.empty((B, T, C), dtype=np.float32)
    for j in range(8):
        out[j // 4, (j % 4) * TQ : (j % 4 + 1) * TQ, :] = res.results[j]["out_t"].T
    kernel._last_exec_time_ns = res.exec_time_ns
    return out


# revision 19
# speedup vs baseline: 1.3309x; 1.0298x over previous
"""Dense transformer block (rmsnorm+causal attention+rope / rmsnorm+SwiGLU) on 8 TRN2 cores.

Sharding (v2):
  core j owns head pair {2j, 2j+1} for BOTH batches (attention head-parallel),
  and owns output row-chunk (b=j//4, rows (j%4)*512..) for phase B (row-parallel).

  Phase A: rmsnorm1 (rstd via all-ones matmul on x^T) -> h^T chunks -> QKV for the
  2 heads x 2 batches -> rope -> causal attention entirely in SBUF.
  Two 8-core AllToAlls reshard head->row: A2A#1 carries head 2j (fires at 50% of
  attention), A2A#2 carries head 2j+1 (fires at the end). Each slice r=(b*4+qc)
  is [128, 512] -> every byte is useful (no cross-batch padding).

  Phase B runs fully transposed (c on partitions, tokens on free dim):
  proj y^T accumulated per 128-col block of C over 16 received head blocks,
  residual from x^T slice, rmsnorm2 via all-ones matmul, SwiGLU with u^T kept
  for all 44 hidden tiles, w3 pass accumulates y3^T per c-block in PSUM.
  Output is written transposed [C, 512]; the host transposes back.

Matmul operands bf16 (weights pre-cast, w_norm folded); stats/PSUM fp32.
"""

import numpy as np
import ml_dtypes

import concourse.bass as bass
import concourse.mybir as mybir
import concourse.tile as tile
from concourse import bacc
from concourse import bass_utils
from concourse.masks import make_identity

AF = mybir.ActivationFunctionType
ALU = mybir.AluOpType
F32 = mybir.dt.float32
BF16 = mybir.dt.bfloat16
MMDT = BF16
NP_MMDT = ml_dtypes.bfloat16

P = 128
T = 2048
C = 2048
D = 128
HD2 = D // 2
H = 16
HPC = 2          # heads per core
B = 2
HID = 5632
HID_T = HID // P  # 44
TQ = 512
CT = C // P      # 16
CHW = 512        # token chunk width in phase A
EPS = 1e-6
ROPE_BASE = 10000.0


def _build():
    nc = bacc.Bacc(None, target_bir_lowering=False, num_devices=8)

    # ---- kernel I/O ----
    x_tb = nc.dram_tensor("x_tb", [B, T // CHW, P, CT, CHW], MMDT, kind="ExternalInput")
    x_res = nc.dram_tensor("x_res", [C, TQ], F32, kind="ExternalInput")
    wq = nc.dram_tensor("wq", [HPC, P, CT, P], MMDT, kind="ExternalInput")
    wk = nc.dram_tensor("wk", [HPC, P, CT, P], MMDT, kind="ExternalInput")
    wv = nc.dram_tensor("wv", [P, CT, HPC * D], MMDT, kind="ExternalInput")
    wpe = nc.dram_tensor("wpe", [2, CT, P, 8, P], MMDT, kind="ExternalInput")
    w1t = nc.dram_tensor("w1t", [HID_T, P, CT, P], MMDT, kind="ExternalInput")
    w2t = nc.dram_tensor("w2t", [HID_T, P, CT, P], MMDT, kind="ExternalInput")
    w3 = nc.dram_tensor("w3", [HID, C], MMDT, kind="ExternalInput")
    rope_t = nc.dram_tensor("rope_t", [D, T], F32, kind="ExternalInput")
    rope_sw = nc.dram_tensor("rope_sw", [D, T], F32, kind="ExternalInput")
    tri = nc.dram_tensor("tri", [P, P], MMDT, kind="ExternalInput")
    out_t = nc.dram_tensor("out_t", [C, TQ], F32, kind="ExternalOutput")

    inv_sqrt_d = 1.0 / float(np.sqrt(D))

    with tile.TileContext(nc) as tc:
        with (
            tc.tile_pool(name="const", bufs=1) as const,
            tc.tile_pool(name="dram", bufs=1, space="DRAM") as dram,
        ):
            # ---- constants (no DMA yet: first x chunk loads first) ----
            ones_f = const.tile([P, P], F32)
            nc.vector.memset(ones_f, 1.0)
            ones128 = const.tile([P, P], MMDT)
            nc.vector.tensor_copy(out=ones128, in_=ones_f)
            ones1 = const.tile([P, 1], MMDT)
            nc.vector.tensor_copy(out=ones1, in_=ones_f[:, 0:1])
            eps_sb = const.tile([P, 1], F32)
            nc.vector.memset(eps_sb, EPS)
            ident_f = const.tile([P, P], F32)
            make_identity(nc, ident_f)
            rope_sb = const.tile([D, T], F32)
            rope_sw_sb = const.tile([D, T], F32)
            tri_sb = const.tile([P, P], MMDT)

            # ---- DRAM scratch for collectives ----
            a2a1_in = dram.tile([8, P, TQ], MMDT)
            a2a1_out = dram.tile([8, P, TQ], MMDT)
            a2a2_in = dram.tile([8, P, TQ], MMDT)
            a2a2_out = dram.tile([8, P, TQ], MMDT)

            # ================= Phase A: QKV + attention =================
            pers_ctx = tc.tile_pool(name="pers", bufs=1)
            pers = pers_ctx.__enter__()
            qT_sb = pers.tile([P, HPC, B, T], MMDT, tag="qT", bufs=1)
            kT_sb = pers.tile([P, HPC, B, T], MMDT, tag="kT", bufs=1)
            v_sb = pers.tile([P, B, T // P, HPC * D], MMDT, tag="v", bufs=1)
            wq_sb = pers.tile([P, HPC, CT, P], MMDT, tag="wq", bufs=1)
            wk_sb = pers.tile([P, HPC, CT, P], MMDT, tag="wk", bufs=1)
            wv_sb = pers.tile([P, CT, HPC * D], MMDT, tag="wv", bufs=1)

            with (
                tc.tile_pool(name="a1", bufs=2) as a1,
                tc.tile_pool(name="a1psum", bufs=2, space="PSUM") as pa1,
            ):
                # first x chunk loads before the weights so PE starts early
                xtc0 = a1.tile([P, CT, CHW], MMDT, tag="xtc", bufs=2, name="xtc0")
                nc.sync.dma_start(out=xtc0, in_=x_tb[0, 0])
                for hl in range(HPC):
                    nc.sync.dma_start(out=wq_sb[:, hl], in_=wq[hl])
                    nc.sync.dma_start(out=wk_sb[:, hl], in_=wk[hl])
                nc.sync.dma_start(out=rope_sb, in_=rope_t[:, :])
                nc.sync.dma_start(out=rope_sw_sb, in_=rope_sw[:, :])
                nc.sync.dma_start(out=wv_sb, in_=wv[:, :, :])
                nc.sync.dma_start(out=tri_sb, in_=tri[:, :])

                for b in range(B):
                    for tci in range(T // CHW):
                        t0 = tci * CHW
                        if b == 0 and tci == 0:
                            xtc = xtc0
                        else:
                            xtc = a1.tile([P, CT, CHW], MMDT, tag="xtc", bufs=2)
                            nc.sync.dma_start(out=xtc, in_=x_tb[b, tci])
                        # rstd for these tokens: colsum of x^2 via ones-matmul.
                        # rstd is a per-token scale, so it commutes through the
                        # QKV contraction: fold into rope tables (q/k) and into
                        # a per-partition scale on v. q/k/v consume raw x.
                        rsp = pa1.tile([1, CHW], F32, tag="rsp", bufs=2)
                        for ct in range(CT):
                            sq = a1.tile([P, CHW], MMDT, tag="sq", bufs=3)
                            nc.vector.tensor_tensor(
                                out=sq, in0=xtc[:, ct, :], in1=xtc[:, ct, :],
                                op=ALU.mult,
                            )
                            nc.tensor.matmul(
                                rsp, ones1, sq, start=(ct == 0), stop=(ct == CT - 1)
                            )
                        rrow = a1.tile([1, CHW], F32, tag="rrow", bufs=2)
                        nc.scalar.activation(
                            rrow, rsp, AF.Sqrt, bias=eps_sb[0:1, :], scale=1.0 / C
                        )
                        rinv = a1.tile([1, CHW], F32, tag="rinv", bufs=2)
                        nc.vector.reciprocal_approx_fast(out=rinv, in_=rrow)
                        rbc = a1.tile([P, CHW], F32, tag="rbc", bufs=2)
                        nc.gpsimd.partition_broadcast(rbc[:], rinv[:])
                        # rcs rows 0:64 = cos*r, 64:128 = sin*r; rcs_sw swapped
                        rcs = a1.tile([D, CHW], F32, tag="rcs", bufs=2)
                        rcs_sw = a1.tile([D, CHW], F32, tag="rcssw", bufs=2)
                        nc.vector.tensor_tensor(
                            out=rcs, in0=rope_sb[:, t0 : t0 + CHW], in1=rbc,
                            op=ALU.mult,
                        )
                        nc.vector.tensor_tensor(
                            out=rcs_sw, in0=rope_sw_sb[:, t0 : t0 + CHW], in1=rbc,
                            op=ALU.mult,
                        )
                        # q^T / k^T from raw x with rstd-folded rope
                        for wsb, dst in ((wq_sb, qT_sb), (wk_sb, kT_sb)):
                            for hl in range(HPC):
                                pq = pa1.tile([P, CHW], F32, tag="pq", bufs=2)
                                for ct in range(CT):
                                    nc.tensor.matmul(
                                        pq,
                                        wsb[:, hl, ct, :],
                                        xtc[:, ct, :],
                                        start=(ct == 0),
                                        stop=(ct == CT - 1),
                                    )
                                x1 = pq[0:HD2, :]
                                x2 = pq[HD2:P, :]
                                tm1 = a1.tile([HD2, CHW], F32, tag="tm1", bufs=2)
                                tm2 = a1.tile([HD2, CHW], F32, tag="tm2", bufs=2)
                                dslc = dst[:, hl, b, t0 : t0 + CHW]
                                nc.vector.tensor_tensor(
                                    out=tm1, in0=x1, in1=rcs[0:HD2, :], op=ALU.mult
                                )
                                nc.vector.tensor_tensor(
                                    out=tm2, in0=x2, in1=rcs[HD2:D, :], op=ALU.mult
                                )
                                nc.vector.tensor_tensor(
                                    out=dslc[0:HD2], in0=tm1, in1=tm2,
                                    op=ALU.subtract,
                                )
                                nc.vector.tensor_tensor(
                                    out=tm1, in0=x1, in1=rcs_sw[0:HD2, :],
                                    op=ALU.mult,
                                )
                                nc.vector.tensor_tensor(
                                    out=tm2, in0=x2, in1=rcs_sw[HD2:D, :],
                                    op=ALU.mult,
                                )
                                nc.vector.tensor_tensor(
                                    out=dslc[HD2:P], in0=tm1, in1=tm2, op=ALU.add
                                )
                        # rstd as per-partition column for the v rows
                        rcols = []
                        for rt in range(CHW // P):
                            rc_ps = pa1.tile([P, 1], F32, tag="rcps", bufs=2)
                            nc.tensor.matmul(
                                rc_ps,
                                rinv[0:1, rt * P : (rt + 1) * P],
                                ones_f[0:1, 0:1],
                                start=True,
                                stop=True,
                            )
                            rcol = a1.tile([P, 1], F32, tag="rcol", bufs=4)
                            nc.scalar.activation(rcol, rc_ps, AF.Copy)
                            rcols.append(rcol)
                        # v rows from raw x, scaled by rstd on eviction
                        for rt in range(CHW // P):
                            pv = pa1.tile([P, HPC * D], F32, tag="pv", bufs=2)
                            for ct in range(CT):
                                nc.tensor.matmul(
                                    pv,
                                    xtc[:, ct, rt * P : (rt + 1) * P],
                                    wv_sb[:, ct, :],
                                    start=(ct == 0),
                                    stop=(ct == CT - 1),
                                )
                            nc.vector.tensor_scalar(
                                out=v_sb[:, b, tci * (CHW // P) + rt, :],
                                in0=pv,
                                scalar1=rcols[rt],
                                scalar2=None,
                                op0=ALU.mult,
                            )

            # ---- causal attention, all in SBUF ----
            with (
                tc.tile_pool(name="att", bufs=2) as att,
                tc.tile_pool(name="attpsum", bufs=2, space="PSUM") as pat,
            ):
                for hl in range(HPC):
                    a2a_in = a2a1_in if hl == 0 else a2a2_in
                    for b in range(B):
                        for qc in range(T // TQ):
                            nkb = 4 * qc + 4
                            l_ps = pat.tile([P, TQ], F32, tag="l", bufs=2)
                            o_ps = pat.tile([P, TQ], F32, tag="o", bufs=2)
                            es = []
                            for kb in range(nkb):
                                r = kb - 4 * qc
                                q0 = max(0, r * P)
                                st = pat.tile([P, TQ], F32, tag="st", bufs=4)
                                nc.tensor.matmul(
                                    st[:, q0:TQ],
                                    kT_sb[:, hl, b, kb * P : (kb + 1) * P],
                                    qT_sb[:, hl, b, qc * TQ + q0 : (qc + 1) * TQ],
                                    start=True,
                                    stop=True,
                                )
                                e = att.tile([P, TQ], MMDT, tag="e", bufs=18)
                                nc.scalar.activation(
                                    e[:, q0:TQ], st[:, q0:TQ], AF.Exp,
                                    scale=inv_sqrt_d,
                                )
                                if r >= 0:
                                    nc.vector.tensor_tensor(
                                        out=e[:, q0 : q0 + P],
                                        in0=e[:, q0 : q0 + P],
                                        in1=tri_sb,
                                        op=ALU.mult,
                                    )
                                es.append((e, q0))
                            for kb in range(nkb):
                                e, q0 = es[kb]
                                nc.tensor.matmul(
                                    l_ps[:, q0:TQ],
                                    ones128,
                                    e[:, q0:TQ],
                                    start=(kb == 0),
                                    stop=(kb == nkb - 1),
                                )
                                nc.tensor.matmul(
                                    o_ps[:, q0:TQ],
                                    v_sb[:, b, kb, hl * D : (hl + 1) * D],
                                    e[:, q0:TQ],
                                    start=(kb == 0),
                                    stop=(kb == nkb - 1),
                                )
                            l_inv = att.tile([P, TQ], F32, tag="linv", bufs=2)
                            nc.vector.reciprocal_approx_fast(out=l_inv, in_=l_ps)
                            oT = att.tile([P, TQ], MMDT, tag="oT", bufs=3)
                            nc.vector.tensor_tensor(
                                out=oT, in0=o_ps, in1=l_inv, op=ALU.mult
                            )
                            nc.sync.dma_start(
                                out=a2a_in[b * 4 + qc, :, :], in_=oT
                            )
                    if hl == 0:
                        nc.gpsimd.collective_compute(
                            "AllToAll",
                            ALU.bypass,
                            replica_groups=[[0, 1, 2, 3, 4, 5, 6, 7]],
                            ins=[a2a1_in.opt()],
                            outs=[a2a1_out.opt()],
                        )
                    else:
                        nc.gpsimd.collective_compute(
                            "AllToAll",
                            ALU.bypass,
                            replica_groups=[[0, 1, 2, 3, 4, 5, 6, 7]],
                            ins=[a2a2_in.opt()],
                            outs=[a2a2_out.opt()],
                        )

            pers_ctx.__exit__(None, None, None)

            # ================= Phase B (transposed) =================
            bres_ctx = tc.tile_pool(name="bres", bufs=1)
            bres = bres_ctx.__enter__()
            xmidT = bres.tile([P, CT, TQ], F32, tag="xmid", bufs=1)
            xmid_bf = bres.tile([P, CT, TQ], MMDT, tag="xmidbf", bufs=1)
            h2T = bres.tile([P, CT, TQ], MMDT, tag="h2T", bufs=1)
            mlp_ctx = tc.tile_pool(name="mlp", bufs=2)
            mlp = mlp_ctx.__enter__()
            # prefetch first MLP weight tiles (DMA runs during attention/proj)
            w12_pref = []
            for ht in range(2):
                w1_p = mlp.tile([P, CT, P], MMDT, tag="w1s", bufs=3, name="w1_p")
                nc.sync.dma_start(out=w1_p, in_=w1t[ht])
                w2_p = mlp.tile([P, CT, P], MMDT, tag="w2s", bufs=3, name="w2_p")
                nc.sync.dma_start(out=w2_p, in_=w2t[ht])
                w12_pref.append((w1_p, w2_p))

            with (
                tc.tile_pool(name="b1", bufs=2) as b1,
                tc.tile_pool(name="b1psum", bufs=2, space="PSUM") as pb1,
            ):
                lp0 = b1.tile([P, 8, TQ], MMDT, tag="lp0", bufs=1)
                nc.sync.dma_start(
                    out=lp0, in_=a2a1_out.rearrange("s p t -> p s t")
                )
                lp1 = b1.tile([P, 8, TQ], MMDT, tag="lp1", bufs=1)
                rsp2 = pb1.tile([1, TQ], F32, tag="rsp2", bufs=1)
                for half in range(2):
                    if half == 1:
                        # emitted after half-0's wpe loads: its A2A#2 wait must
                        # not block them in the sync-engine DMA FIFO
                        nc.sync.dma_start(
                            out=lp1, in_=a2a2_out.rearrange("s p t -> p s t")
                        )
                    lp = lp0 if half == 0 else lp1
                    for ctb in range(CT):
                        wpet = b1.tile([P, 8, P], MMDT, tag="wpet", bufs=3)
                        nc.sync.dma_start(out=wpet, in_=wpe[half, ctb])
                        if half == 0:
                            xres_t = b1.tile([P, TQ], F32, tag="xrest", bufs=3)
                            nc.sync.dma_start(
                                out=xres_t, in_=x_res[ctb * P : (ctb + 1) * P, :]
                            )
                        yps = pb1.tile([P, TQ], F32, tag="yps", bufs=2)
                        for s in range(8):
                            nc.tensor.matmul(
                                yps,
                                wpet[:, s, :],
                                lp[:, s, :],
                                start=(s == 0),
                                stop=(s == 7),
                            )
                        if half == 0:
                            nc.vector.tensor_tensor(
                                out=xmidT[:, ctb, :], in0=yps, in1=xres_t,
                                op=ALU.add,
                            )
                        else:
                            nc.vector.tensor_tensor(
                                out=xmidT[:, ctb, :], in0=yps,
                                in1=xmidT[:, ctb, :], op=ALU.add,
                            )
                            nc.vector.tensor_copy(
                                out=xmid_bf[:, ctb, :], in_=xmidT[:, ctb, :]
                            )
                            sq2 = b1.tile([P, TQ], MMDT, tag="sq2", bufs=3)
                            nc.vector.tensor_tensor(
                                out=sq2, in0=xmidT[:, ctb, :],
                                in1=xmidT[:, ctb, :], op=ALU.mult,
                            )
                            nc.tensor.matmul(
                                rsp2, ones1, sq2,
                                start=(ctb == 0), stop=(ctb == CT - 1),
                            )
                rrow2 = b1.tile([1, TQ], F32, tag="rrow2", bufs=1)
                nc.scalar.activation(
                    rrow2, rsp2, AF.Sqrt, bias=eps_sb[0:1, :], scale=1.0 / C
                )
                rinv2 = b1.tile([1, TQ], F32, tag="rinv2", bufs=1)
                nc.vector.reciprocal_approx_fast(out=rinv2, in_=rrow2)
                rinv2b = b1.tile([1, TQ], MMDT, tag="rinv2b", bufs=1)
                nc.vector.tensor_copy(out=rinv2b, in_=rinv2)
                rbc2 = b1.tile([P, TQ], MMDT, tag="rbc2", bufs=1)
                nc.gpsimd.partition_broadcast(rbc2[:], rinv2b[:])
                for ctb in range(CT):
                    nc.vector.tensor_tensor(
                        out=h2T[:, ctb, :], in0=xmid_bf[:, ctb, :], in1=rbc2,
                        op=ALU.mult,
                    )

            # ---- SwiGLU MLP ----
            if True:
                uT = mlp.tile([P, HID_T, TQ], MMDT, tag="uT", bufs=1)
                with tc.tile_pool(name="mlpg", bufs=2, space="PSUM") as pg:
                    for ht in range(HID_T):
                        if ht < len(w12_pref):
                            w1_sb, w2_sb = w12_pref[ht]
                        else:
                            w1_sb = mlp.tile([P, CT, P], MMDT, tag="w1s", bufs=3)
                            nc.sync.dma_start(out=w1_sb, in_=w1t[ht])
                            w2_sb = mlp.tile([P, CT, P], MMDT, tag="w2s", bufs=3)
                            nc.sync.dma_start(out=w2_sb, in_=w2t[ht])
                        g1 = pg.tile([P, TQ], F32, tag="g1", bufs=2)
                        g2 = pg.tile([P, TQ], F32, tag="g2", bufs=2)
                        for ct in range(CT):
                            nc.tensor.matmul(
                                g1, w1_sb[:, ct, :], h2T[:, ct, :],
                                start=(ct == 0), stop=(ct == CT - 1),
                            )
                        for ct in range(CT):
                            nc.tensor.matmul(
                                g2, w2_sb[:, ct, :], h2T[:, ct, :],
                                start=(ct == 0), stop=(ct == CT - 1),
                            )
                        sil = mlp.tile([P, TQ], F32, tag="sil", bufs=3)
                        nc.scalar.activation(sil, g1, AF.Silu)
                        nc.vector.tensor_tensor(
                            out=uT[:, ht, :], in0=g2, in1=sil, op=ALU.mult
                        )
                with tc.tile_pool(name="mlpy", bufs=1, space="PSUM") as py:
                    for g in range(2):
                        y3ps = [
                            py.tile(
                                [P, TQ], F32, tag=f"y3_{ci}", bufs=1,
                                name=f"y3_{ci}",
                            )
                            for ci in range(8)
                        ]
                        for ht in range(HID_T):
                            w3_sb = mlp.tile([P, 8 * P], MMDT, tag="w3s", bufs=3)
                            nc.sync.dma_start(
                                out=w3_sb,
                                in_=w3[
                                    ht * P : (ht + 1) * P,
                                    g * 8 * P : (g + 1) * 8 * P,
                                ],
                            )
                            for ci in range(8):
                                nc.tensor.matmul(
                                    y3ps[ci],
                                    w3_sb[:, ci * P : (ci + 1) * P],
                                    uT[:, ht, :],
                                    start=(ht == 0),
                                    stop=(ht == HID_T - 1),
                                )
                        for ci in range(8):
                            ctb = g * 8 + ci
                            of = mlp.tile([P, TQ], F32, tag="of", bufs=3)
                            nc.vector.tensor_tensor(
                                out=of, in0=y3ps[ci], in1=xmidT[:, ctb, :],
                                op=ALU.add,
                            )
                            nc.sync.dma_start(
                                out=out_t[ctb * P : (ctb + 1) * P, :], in_=of
                            )

            mlp_ctx.__exit__(None, None, None)
            bres_ctx.__exit__(None, None, None)

    nc.compile()
    return nc


_NC_CACHE = None


def _get_nc():
    global _NC_CACHE
    if _NC_CACHE is None:
        _NC_CACHE = _build()
    return _NC_CACHE


def _host_inputs(x, w_norm1, w_qkv, w_proj, w_norm2, w1, w2, w3):
    x = np.asarray(x, dtype=np.float32)
    w_qkv = np.asarray(w_qkv, dtype=np.float32)
    w_proj = np.asarray(w_proj, dtype=np.float32)
    w_norm1 = np.asarray(w_norm1, dtype=np.float32)
    w_norm2 = np.asarray(w_norm2, dtype=np.float32)
    w1 = np.asarray(w1, dtype=np.float32)
    w2 = np.asarray(w2, dtype=np.float32)
    w3 = np.asarray(w3, dtype=np.float32)

    inv_freq = 1.0 / (ROPE_BASE ** (np.arange(HD2, dtype=np.float32) / HD2))
    pos = np.arange(T, dtype=np.float32)
    freqs = pos[:, None] * inv_freq[None, :]
    rope_tab = np.ascontiguousarray(
        np.concatenate([np.cos(freqs).T, np.sin(freqs).T], axis=0).astype(np.float32)
    )
    rope_tab_sw = np.ascontiguousarray(
        np.concatenate([np.sin(freqs).T, np.cos(freqs).T], axis=0).astype(np.float32)
    )

    ql = np.arange(P)[None, :]
    kv = np.arange(P)[:, None]
    tri = (ql >= kv).astype(NP_MMDT)

    # fold w_norm into weight rows (h @ W == (x*rstd) @ (diag(wn) W))
    w_qkv_n = w_qkv * w_norm1[:, None]
    w1_n = w1 * w_norm2[:, None]
    w2_n = w2 * w_norm2[:, None]

    # shared across cores; all streamed tiles made contiguous in DRAM
    x_t32 = np.ascontiguousarray(x.transpose(0, 2, 1))
    x_tb = np.ascontiguousarray(
        x_t32.reshape(B, CT, P, T // CHW, CHW).transpose(0, 3, 2, 1, 4)
    ).astype(NP_MMDT)
    w1t = np.ascontiguousarray(
        w1_n.reshape(CT, P, HID_T, P).transpose(2, 1, 0, 3)
    ).astype(NP_MMDT)
    w2t = np.ascontiguousarray(
        w2_n.reshape(CT, P, HID_T, P).transpose(2, 1, 0, 3)
    ).astype(NP_MMDT)
    w3_b = w3.astype(NP_MMDT)
    # wpe row block (k*8+s) <- w_proj rows of head 2s+k; then [hf,ctb,p,blk,c]
    perm = np.empty(16, dtype=np.int64)
    for k in range(2):
        for s in range(8):
            perm[k * 8 + s] = 2 * s + k
    wpe_cc = w_proj.reshape(16, P, C)[perm].reshape(C, C)
    wpe = np.ascontiguousarray(
        wpe_cc.reshape(2, 8, P, CT, P).transpose(0, 3, 2, 1, 4)
    ).astype(NP_MMDT)

    in_maps = []
    for j in range(8):
        b, hg = j // 4, j % 4
        col0 = 2 * j * D
        in_maps.append(
            {
                "x_tb": x_tb,
                "x_res": np.ascontiguousarray(
                    x_t32[b, :, hg * TQ : (hg + 1) * TQ]
                ),
                "wq": np.ascontiguousarray(
                    w_qkv_n[:, col0 : col0 + HPC * D]
                    .reshape(CT, P, HPC, D).transpose(2, 1, 0, 3)
                ).astype(NP_MMDT),
                "wk": np.ascontiguousarray(
                    w_qkv_n[:, C + col0 : C + col0 + HPC * D]
                    .reshape(CT, P, HPC, D).transpose(2, 1, 0, 3)
                ).astype(NP_MMDT),
                "wv": np.ascontiguousarray(
                    w_qkv_n[:, 2 * C + col0 : 2 * C + col0 + HPC * D]
                    .reshape(CT, P, HPC * D).transpose(1, 0, 2)
                ).astype(NP_MMDT),
                "wpe": wpe,
                "w1t": w1t,
                "w2t": w2t,
                "w3": w3_b,
                "rope_t": rope_tab,
                "rope_sw": rope_tab_sw,
                "tri": tri,
            }
        )
    return in_maps


def kernel(x, w_norm1, w_qkv, w_proj, w_norm2, w1, w2, w3, _trace=False, _tmpdir=None):
    nc = _get_nc()
    in_maps = _host_inputs(x, w_norm1, w_qkv, w_proj, w_norm2, w1, w2, w3)
    kwargs = {}
    if _trace:
        kwargs = {"trace": True, "tmpdir": _tmpdir}
    res = bass_utils.run_bass_kernel_spmd(
        nc, in_maps, core_ids=list(range(8)), **kwargs
    )
    out = np.empty((B, T, C), dtype=np.float32)
    for j in range(8):
        out[j // 4, (j % 4) * TQ : (j % 4 + 1) * TQ, :] = res.results[j]["out_t"].T
    kernel._last_exec_time_ns = res.exec_time_ns
    return out


# revision 20
# speedup vs baseline: 1.3492x; 1.0138x over previous
"""Dense transformer block (rmsnorm+causal attention+rope / rmsnorm+SwiGLU) on 8 TRN2 cores.

Sharding (v2):
  core j owns head pair {2j, 2j+1} for BOTH batches (attention head-parallel),
  and owns output row-chunk (b=j//4, rows (j%4)*512..) for phase B (row-parallel).

  Phase A: rmsnorm1 (rstd via all-ones matmul on x^T) -> h^T chunks -> QKV for the
  2 heads x 2 batches -> rope -> causal attention entirely in SBUF.
  Two 8-core AllToAlls reshard head->row: A2A#1 carries head 2j (fires at 50% of
  attention), A2A#2 carries head 2j+1 (fires at the end). Each slice r=(b*4+qc)
  is [128, 512] -> every byte is useful (no cross-batch padding).

  Phase B runs fully transposed (c on partitions, tokens on free dim):
  proj y^T accumulated per 128-col block of C over 16 received head blocks,
  residual from x^T slice, rmsnorm2 via all-ones matmul, SwiGLU with u^T kept
  for all 44 hidden tiles, w3 pass accumulates y3^T per c-block in PSUM.
  Output is written transposed [C, 512]; the host transposes back.

Matmul operands bf16 (weights pre-cast, w_norm folded); stats/PSUM fp32.
"""

import numpy as np
import ml_dtypes

import concourse.bass as bass
import concourse.mybir as mybir
import concourse.tile as tile
from concourse import bacc
from concourse import bass_utils
from concourse.masks import make_identity

AF = mybir.ActivationFunctionType
ALU = mybir.AluOpType
F32 = mybir.dt.float32
BF16 = mybir.dt.bfloat16
MMDT = BF16
NP_MMDT = ml_dtypes.bfloat16

P = 128
T = 2048
C = 2048
D = 128
HD2 = D // 2
H = 16
HPC = 2          # heads per core
B = 2
HID = 5632
HID_T = HID // P  # 44
TQ = 512
CT = C // P      # 16
CHW = 512        # token chunk width in phase A
EPS = 1e-6
ROPE_BASE = 10000.0


def _build():
    nc = bacc.Bacc(None, target_bir_lowering=False, num_devices=8)

    # ---- kernel I/O ----
    x_tb = nc.dram_tensor("x_tb", [B, T // CHW, P, CT, CHW], MMDT, kind="ExternalInput")
    x_res = nc.dram_tensor("x_res", [C, TQ], F32, kind="ExternalInput")
    wq = nc.dram_tensor("wq", [HPC, P, CT, P], MMDT, kind="ExternalInput")
    wk = nc.dram_tensor("wk", [HPC, P, CT, P], MMDT, kind="ExternalInput")
    wv = nc.dram_tensor("wv", [P, CT, HPC * D], MMDT, kind="ExternalInput")
    wpe = nc.dram_tensor("wpe", [2, CT, P, 8, P], MMDT, kind="ExternalInput")
    w1t = nc.dram_tensor("w1t", [HID_T, P, CT, P], MMDT, kind="ExternalInput")
    w2t = nc.dram_tensor("w2t", [HID_T, P, CT, P], MMDT, kind="ExternalInput")
    w3 = nc.dram_tensor("w3", [HID, C], MMDT, kind="ExternalInput")
    rope_t = nc.dram_tensor("rope_t", [D, T], F32, kind="ExternalInput")
    rope_sw = nc.dram_tensor("rope_sw", [D, T], F32, kind="ExternalInput")
    tri = nc.dram_tensor("tri", [P, P], MMDT, kind="ExternalInput")
    out_t = nc.dram_tensor("out_t", [C, TQ], F32, kind="ExternalOutput")

    inv_sqrt_d = 1.0 / float(np.sqrt(D))

    with tile.TileContext(nc) as tc:
        with (
            tc.tile_pool(name="const", bufs=1) as const,
            tc.tile_pool(name="dram", bufs=1, space="DRAM") as dram,
        ):
            # ---- constants (no DMA yet: first x chunk loads first) ----
            ones_f = const.tile([P, P], F32)
            nc.vector.memset(ones_f, 1.0)
            ones128 = const.tile([P, P], MMDT)
            nc.vector.tensor_copy(out=ones128, in_=ones_f)
            ones1 = const.tile([P, 1], MMDT)
            nc.vector.tensor_copy(out=ones1, in_=ones_f[:, 0:1])
            eps_sb = const.tile([P, 1], F32)
            nc.vector.memset(eps_sb, EPS)
            ident_f = const.tile([P, P], F32)
            make_identity(nc, ident_f)
            rope_sb = const.tile([D, T], F32)
            rope_sw_sb = const.tile([D, T], F32)
            tri_sb = const.tile([P, P], MMDT)

            # ---- DRAM scratch for collectives ----
            a2a1_in = dram.tile([8, P, TQ], MMDT)
            a2a1_out = dram.tile([8, P, TQ], MMDT)
            a2a2_in = dram.tile([8, P, TQ], MMDT)
            a2a2_out = dram.tile([8, P, TQ], MMDT)

            # ================= Phase A: QKV + attention =================
            pers_ctx = tc.tile_pool(name="pers", bufs=1)
            pers = pers_ctx.__enter__()
            qT_sb = pers.tile([P, HPC, B, T], MMDT, tag="qT", bufs=1)
            kT_sb = pers.tile([P, HPC, B, T], MMDT, tag="kT", bufs=1)
            v_sb = pers.tile([P, B, T // P, HPC * D], MMDT, tag="v", bufs=1)
            wq_sb = pers.tile([P, HPC, CT, P], MMDT, tag="wq", bufs=1)
            wk_sb = pers.tile([P, HPC, CT, P], MMDT, tag="wk", bufs=1)
            wv_sb = pers.tile([P, CT, HPC * D], MMDT, tag="wv", bufs=1)

            with (
                tc.tile_pool(name="a1", bufs=2) as a1,
                tc.tile_pool(name="a1psum", bufs=2, space="PSUM") as pa1,
            ):
                # first x chunk loads before the weights so PE starts early
                xtc0 = a1.tile([P, CT, CHW], MMDT, tag="xtc", bufs=2, name="xtc0")
                nc.sync.dma_start(out=xtc0, in_=x_tb[0, 0])
                for hl in range(HPC):
                    nc.sync.dma_start(out=wq_sb[:, hl], in_=wq[hl])
                    nc.sync.dma_start(out=wk_sb[:, hl], in_=wk[hl])
                nc.sync.dma_start(out=rope_sb, in_=rope_t[:, :])
                nc.sync.dma_start(out=rope_sw_sb, in_=rope_sw[:, :])
                nc.sync.dma_start(out=wv_sb, in_=wv[:, :, :])
                nc.sync.dma_start(out=tri_sb, in_=tri[:, :])

                for b in range(B):
                    for tci in range(T // CHW):
                        t0 = tci * CHW
                        if b == 0 and tci == 0:
                            xtc = xtc0
                        else:
                            xtc = a1.tile([P, CT, CHW], MMDT, tag="xtc", bufs=2)
                            nc.sync.dma_start(out=xtc, in_=x_tb[b, tci])
                        # rstd for these tokens: colsum of x^2 via ones-matmul.
                        # rstd is a per-token scale, so it commutes through the
                        # QKV contraction: fold into rope tables (q/k) and into
                        # a per-partition scale on v. q/k/v consume raw x.
                        rsp = pa1.tile([1, CHW], F32, tag="rsp", bufs=2)
                        for ct in range(CT):
                            sq = a1.tile([P, CHW], MMDT, tag="sq", bufs=3)
                            nc.vector.tensor_tensor(
                                out=sq, in0=xtc[:, ct, :], in1=xtc[:, ct, :],
                                op=ALU.mult,
                            )
                            nc.tensor.matmul(
                                rsp, ones1, sq, start=(ct == 0), stop=(ct == CT - 1)
                            )
                        rrow = a1.tile([1, CHW], F32, tag="rrow", bufs=2)
                        nc.scalar.activation(
                            rrow, rsp, AF.Sqrt, bias=eps_sb[0:1, :], scale=1.0 / C
                        )
                        rinv = a1.tile([1, CHW], F32, tag="rinv", bufs=2)
                        nc.vector.reciprocal_approx_fast(out=rinv, in_=rrow)
                        rbc = a1.tile([P, CHW], F32, tag="rbc", bufs=2)
                        nc.gpsimd.partition_broadcast(rbc[:], rinv[:])
                        # rcs rows 0:64 = cos*r, 64:128 = sin*r; rcs_sw swapped
                        rcs = a1.tile([D, CHW], F32, tag="rcs", bufs=2)
                        rcs_sw = a1.tile([D, CHW], F32, tag="rcssw", bufs=2)
                        nc.vector.tensor_tensor(
                            out=rcs, in0=rope_sb[:, t0 : t0 + CHW], in1=rbc,
                            op=ALU.mult,
                        )
                        nc.vector.tensor_tensor(
                            out=rcs_sw, in0=rope_sw_sb[:, t0 : t0 + CHW], in1=rbc,
                            op=ALU.mult,
                        )
                        # q^T / k^T from raw x with rstd-folded rope
                        for wsb, dst in ((wq_sb, qT_sb), (wk_sb, kT_sb)):
                            for hl in range(HPC):
                                pq = pa1.tile([P, CHW], F32, tag="pq", bufs=2)
                                for ct in range(CT):
                                    nc.tensor.matmul(
                                        pq,
                                        wsb[:, hl, ct, :],
                                        xtc[:, ct, :],
                                        start=(ct == 0),
                                        stop=(ct == CT - 1),
                                    )
                                x1 = pq[0:HD2, :]
                                x2 = pq[HD2:P, :]
                                tm1 = a1.tile([HD2, CHW], F32, tag="tm1", bufs=2)
                                tm2 = a1.tile([HD2, CHW], F32, tag="tm2", bufs=2)
                                dslc = dst[:, hl, b, t0 : t0 + CHW]
                                nc.vector.tensor_tensor(
                                    out=tm1, in0=x1, in1=rcs[0:HD2, :], op=ALU.mult
                                )
                                nc.vector.tensor_tensor(
                                    out=tm2, in0=x2, in1=rcs[HD2:D, :], op=ALU.mult
                                )
                                nc.vector.tensor_tensor(
                                    out=dslc[0:HD2], in0=tm1, in1=tm2,
                                    op=ALU.subtract,
                                )
                                nc.vector.tensor_tensor(
                                    out=tm1, in0=x1, in1=rcs_sw[0:HD2, :],
                                    op=ALU.mult,
                                )
                                nc.vector.tensor_tensor(
                                    out=tm2, in0=x2, in1=rcs_sw[HD2:D, :],
                                    op=ALU.mult,
                                )
                                nc.vector.tensor_tensor(
                                    out=dslc[HD2:P], in0=tm1, in1=tm2, op=ALU.add
                                )
                        # rstd as per-partition column for the v rows
                        rcols = []
                        for rt in range(CHW // P):
                            rc_ps = pa1.tile([P, 1], F32, tag="rcps", bufs=2)
                            nc.tensor.matmul(
                                rc_ps,
                                rinv[0:1, rt * P : (rt + 1) * P],
                                ones_f[0:1, 0:1],
                                start=True,
                                stop=True,
                            )
                            rcol = a1.tile([P, 1], F32, tag="rcol", bufs=4)
                            nc.scalar.activation(rcol, rc_ps, AF.Copy)
                            rcols.append(rcol)
                        # v rows from raw x, scaled by rstd on eviction
                        for rt in range(CHW // P):
                            pv = pa1.tile([P, HPC * D], F32, tag="pv", bufs=2)
                            for ct in range(CT):
                                nc.tensor.matmul(
                                    pv,
                                    xtc[:, ct, rt * P : (rt + 1) * P],
                                    wv_sb[:, ct, :],
                                    start=(ct == 0),
                                    stop=(ct == CT - 1),
                                )
                            nc.vector.tensor_scalar(
                                out=v_sb[:, b, tci * (CHW // P) + rt, :],
                                in0=pv,
                                scalar1=rcols[rt],
                                scalar2=None,
                                op0=ALU.mult,
                            )

            # ---- causal attention, all in SBUF ----
            with (
                tc.tile_pool(name="att", bufs=2) as att,
                tc.tile_pool(name="attpsum", bufs=2, space="PSUM") as pat,
            ):
                for hl in range(HPC):
                    a2a_in = a2a1_in if hl == 0 else a2a2_in
                    for b in range(B):
                        for qc in range(T // TQ):
                            nkb = 4 * qc + 4
                            l_ps = pat.tile([P, TQ], F32, tag="l", bufs=2)
                            o_ps = pat.tile([P, TQ], F32, tag="o", bufs=2)
                            es = []
                            for kb in range(nkb):
                                r = kb - 4 * qc
                                q0 = max(0, r * P)
                                st = pat.tile([P, TQ], F32, tag="st", bufs=4)
                                nc.tensor.matmul(
                                    st[:, q0:TQ],
                                    kT_sb[:, hl, b, kb * P : (kb + 1) * P],
                                    qT_sb[:, hl, b, qc * TQ + q0 : (qc + 1) * TQ],
                                    start=True,
                                    stop=True,
                                )
                                e = att.tile([P, TQ], MMDT, tag="e", bufs=18)
                                nc.scalar.activation(
                                    e[:, q0:TQ], st[:, q0:TQ], AF.Exp,
                                    scale=inv_sqrt_d,
                                )
                                if r >= 0:
                                    nc.vector.tensor_tensor(
                                        out=e[:, q0 : q0 + P],
                                        in0=e[:, q0 : q0 + P],
                                        in1=tri_sb,
                                        op=ALU.mult,
                                    )
                                es.append((e, q0))
                            for kb in range(nkb):
                                e, q0 = es[kb]
                                nc.tensor.matmul(
                                    l_ps[:, q0:TQ],
                                    ones128,
                                    e[:, q0:TQ],
                                    start=(kb == 0),
                                    stop=(kb == nkb - 1),
                                )
                            for kb in range(nkb):
                                e, q0 = es[kb]
                                nc.tensor.matmul(
                                    o_ps[:, q0:TQ],
                                    v_sb[:, b, kb, hl * D : (hl + 1) * D],
                                    e[:, q0:TQ],
                                    start=(kb == 0),
                                    stop=(kb == nkb - 1),
                                )
                            l_inv = att.tile([P, TQ], F32, tag="linv", bufs=2)
                            nc.vector.reciprocal_approx_fast(out=l_inv, in_=l_ps)
                            oT = att.tile([P, TQ], MMDT, tag="oT", bufs=3)
                            nc.vector.tensor_tensor(
                                out=oT, in0=o_ps, in1=l_inv, op=ALU.mult
                            )
                            nc.sync.dma_start(
                                out=a2a_in[b * 4 + qc, :, :], in_=oT
                            )
                    if hl == 0:
                        nc.gpsimd.collective_compute(
                            "AllToAll",
                            ALU.bypass,
                            replica_groups=[[0, 1, 2, 3, 4, 5, 6, 7]],
                            ins=[a2a1_in.opt()],
                            outs=[a2a1_out.opt()],
                        )
                    else:
                        nc.gpsimd.collective_compute(
                            "AllToAll",
                            ALU.bypass,
                            replica_groups=[[0, 1, 2, 3, 4, 5, 6, 7]],
                            ins=[a2a2_in.opt()],
                            outs=[a2a2_out.opt()],
                        )

            pers_ctx.__exit__(None, None, None)

            # ================= Phase B (transposed) =================
            bres_ctx = tc.tile_pool(name="bres", bufs=1)
            bres = bres_ctx.__enter__()
            xmidT = bres.tile([P, CT, TQ], F32, tag="xmid", bufs=1)
            xmid_bf = bres.tile([P, CT, TQ], MMDT, tag="xmidbf", bufs=1)
            h2Ts = [
                bres.tile([P, TQ], MMDT, tag=f"h2t{ct}", bufs=1, name=f"h2t{ct}")
                for ct in range(CT)
            ]
            mlp_ctx = tc.tile_pool(name="mlp", bufs=2)
            mlp = mlp_ctx.__enter__()
            # prefetch first MLP weight tiles (DMA runs during attention/proj)
            w12_pref = []
            for ht in range(2):
                w1_p = mlp.tile([P, CT, P], MMDT, tag="w1s", bufs=3, name="w1_p")
                nc.sync.dma_start(out=w1_p, in_=w1t[ht])
                w2_p = mlp.tile([P, CT, P], MMDT, tag="w2s", bufs=3, name="w2_p")
                nc.sync.dma_start(out=w2_p, in_=w2t[ht])
                w12_pref.append((w1_p, w2_p))

            with (
                tc.tile_pool(name="b1", bufs=2) as b1,
                tc.tile_pool(name="b1psum", bufs=2, space="PSUM") as pb1,
            ):
                lp0 = b1.tile([P, 8, TQ], MMDT, tag="lp0", bufs=1)
                nc.sync.dma_start(
                    out=lp0, in_=a2a1_out.rearrange("s p t -> p s t")
                )
                lp1 = b1.tile([P, 8, TQ], MMDT, tag="lp1", bufs=1)
                rsp2 = pb1.tile([1, TQ], F32, tag="rsp2", bufs=1)
                for half in range(2):
                    if half == 1:
                        # emitted after half-0's wpe loads: its A2A#2 wait must
                        # not block them in the sync-engine DMA FIFO
                        nc.sync.dma_start(
                            out=lp1, in_=a2a2_out.rearrange("s p t -> p s t")
                        )
                    lp = lp0 if half == 0 else lp1
                    for ctb in range(CT):
                        wpet = b1.tile([P, 8, P], MMDT, tag="wpet", bufs=3)
                        nc.sync.dma_start(out=wpet, in_=wpe[half, ctb])
                        if half == 0:
                            xres_t = b1.tile([P, TQ], F32, tag="xrest", bufs=3)
                            nc.sync.dma_start(
                                out=xres_t, in_=x_res[ctb * P : (ctb + 1) * P, :]
                            )
                        yps = pb1.tile([P, TQ], F32, tag="yps", bufs=4)
                        for s in range(8):
                            nc.tensor.matmul(
                                yps,
                                wpet[:, s, :],
                                lp[:, s, :],
                                start=(s == 0),
                                stop=(s == 7),
                            )
                        if half == 0:
                            nc.vector.tensor_tensor(
                                out=xmidT[:, ctb, :], in0=yps, in1=xres_t,
                                op=ALU.add,
                            )
                        else:
                            nc.vector.tensor_tensor(
                                out=xmidT[:, ctb, :], in0=yps,
                                in1=xmidT[:, ctb, :], op=ALU.add,
                            )
                            nc.vector.tensor_copy(
                                out=xmid_bf[:, ctb, :], in_=xmidT[:, ctb, :]
                            )
                            sq2 = b1.tile([P, TQ], MMDT, tag="sq2", bufs=3)
                            nc.vector.tensor_tensor(
                                out=sq2, in0=xmidT[:, ctb, :],
                                in1=xmidT[:, ctb, :], op=ALU.mult,
                            )
                            nc.tensor.matmul(
                                rsp2, ones1, sq2,
                                start=(ctb == 0), stop=(ctb == CT - 1),
                            )
                rrow2 = b1.tile([1, TQ], F32, tag="rrow2", bufs=1)
                nc.scalar.activation(
                    rrow2, rsp2, AF.Sqrt, bias=eps_sb[0:1, :], scale=1.0 / C
                )
                rinv2 = b1.tile([1, TQ], F32, tag="rinv2", bufs=1)
                nc.vector.reciprocal_approx_fast(out=rinv2, in_=rrow2)
                rinv2b = b1.tile([1, TQ], MMDT, tag="rinv2b", bufs=1)
                nc.vector.tensor_copy(out=rinv2b, in_=rinv2)
                rbc2 = b1.tile([P, TQ], MMDT, tag="rbc2", bufs=1)
                nc.gpsimd.partition_broadcast(rbc2[:], rinv2b[:])
                for ctb in range(CT):
                    nc.vector.tensor_tensor(
                        out=h2Ts[ctb], in0=xmid_bf[:, ctb, :], in1=rbc2,
                        op=ALU.mult,
                    )

            # ---- SwiGLU MLP ----
            if True:
                uT = mlp.tile([P, HID_T, TQ], MMDT, tag="uT", bufs=1)
                with tc.tile_pool(name="mlpg", bufs=2, space="PSUM") as pg:
                    for ht in range(HID_T):
                        if ht < len(w12_pref):
                            w1_sb, w2_sb = w12_pref[ht]
                        else:
                            w1_sb = mlp.tile([P, CT, P], MMDT, tag="w1s", bufs=3)
                            nc.sync.dma_start(out=w1_sb, in_=w1t[ht])
                            w2_sb = mlp.tile([P, CT, P], MMDT, tag="w2s", bufs=3)
                            nc.sync.dma_start(out=w2_sb, in_=w2t[ht])
                        g1 = pg.tile([P, TQ], F32, tag="g1", bufs=2)
                        g2 = pg.tile([P, TQ], F32, tag="g2", bufs=2)
                        for ct in range(CT):
                            nc.tensor.matmul(
                                g1, w1_sb[:, ct, :], h2Ts[ct],
                                start=(ct == 0), stop=(ct == CT - 1),
                            )
                        for ct in range(CT):
                            nc.tensor.matmul(
                                g2, w2_sb[:, ct, :], h2Ts[ct],
                                start=(ct == 0), stop=(ct == CT - 1),
                            )
                        sil = mlp.tile([P, TQ], F32, tag="sil", bufs=3)
                        nc.scalar.activation(sil, g1, AF.Silu)
                        nc.vector.tensor_tensor(
                            out=uT[:, ht, :], in0=g2, in1=sil, op=ALU.mult
                        )
                with tc.tile_pool(name="mlpy", bufs=1, space="PSUM") as py:
                    for g in range(2):
                        y3ps = [
                            py.tile(
                                [P, TQ], F32, tag=f"y3_{ci}", bufs=1,
                                name=f"y3_{ci}",
                            )
                            for ci in range(8)
                        ]
                        for ht in range(HID_T):
                            w3_sb = mlp.tile([P, 8 * P], MMDT, tag="w3s", bufs=3)
                            nc.sync.dma_start(
                                out=w3_sb,
                                in_=w3[
                                    ht * P : (ht + 1) * P,
                                    g * 8 * P : (g + 1) * 8 * P,
                                ],
                            )
                            for ci in range(8):
                                nc.tensor.matmul(
                                    y3ps[ci],
                                    w3_sb[:, ci * P : (ci + 1) * P],
                                    uT[:, ht, :],
                                    start=(ht == 0),
                                    stop=(ht == HID_T - 1),
                                )
                        for ci in range(8):
                            ctb = g * 8 + ci
                            of = mlp.tile([P, TQ], F32, tag="of", bufs=3)
                            nc.vector.tensor_tensor(
                                out=of, in0=y3ps[ci], in1=xmidT[:, ctb, :],
                                op=ALU.add,
                            )
                            nc.sync.dma_start(
                                out=out_t[ctb * P : (ctb + 1) * P, :], in_=of
                            )

            mlp_ctx.__exit__(None, None, None)
            bres_ctx.__exit__(None, None, None)

    nc.compile()
    return nc


_NC_CACHE = None


def _get_nc():
    global _NC_CACHE
    if _NC_CACHE is None:
        _NC_CACHE = _build()
    return _NC_CACHE


def _host_inputs(x, w_norm1, w_qkv, w_proj, w_norm2, w1, w2, w3):
    x = np.asarray(x, dtype=np.float32)
    w_qkv = np.asarray(w_qkv, dtype=np.float32)
    w_proj = np.asarray(w_proj, dtype=np.float32)
    w_norm1 = np.asarray(w_norm1, dtype=np.float32)
    w_norm2 = np.asarray(w_norm2, dtype=np.float32)
    w1 = np.asarray(w1, dtype=np.float32)
    w2 = np.asarray(w2, dtype=np.float32)
    w3 = np.asarray(w3, dtype=np.float32)

    inv_freq = 1.0 / (ROPE_BASE ** (np.arange(HD2, dtype=np.float32) / HD2))
    pos = np.arange(T, dtype=np.float32)
    freqs = pos[:, None] * inv_freq[None, :]
    rope_tab = np.ascontiguousarray(
        np.concatenate([np.cos(freqs).T, np.sin(freqs).T], axis=0).astype(np.float32)
    )
    rope_tab_sw = np.ascontiguousarray(
        np.concatenate([np.sin(freqs).T, np.cos(freqs).T], axis=0).astype(np.float32)
    )

    ql = np.arange(P)[None, :]
    kv = np.arange(P)[:, None]
    tri = (ql >= kv).astype(NP_MMDT)

    # fold w_norm into weight rows (h @ W == (x*rstd) @ (diag(wn) W))
    w_qkv_n = w_qkv * w_norm1[:, None]
    w1_n = w1 * w_norm2[:, None]
    w2_n = w2 * w_norm2[:, None]

    # shared across cores; all streamed tiles made contiguous in DRAM
    x_t32 = np.ascontiguousarray(x.transpose(0, 2, 1))
    x_tb = np.ascontiguousarray(
        x_t32.reshape(B, CT, P, T // CHW, CHW).transpose(0, 3, 2, 1, 4)
    ).astype(NP_MMDT)
    w1t = np.ascontiguousarray(
        w1_n.reshape(CT, P, HID_T, P).transpose(2, 1, 0, 3)
    ).astype(NP_MMDT)
    w2t = np.ascontiguousarray(
        w2_n.reshape(CT, P, HID_T, P).transpose(2, 1, 0, 3)
    ).astype(NP_MMDT)
    w3_b = w3.astype(NP_MMDT)
    # wpe row block (k*8+s) <- w_proj rows of head 2s+k; then [hf,ctb,p,blk,c]
    perm = np.empty(16, dtype=np.int64)
    for k in range(2):
        for s in range(8):
            perm[k * 8 + s] = 2 * s + k
    wpe_cc = w_proj.reshape(16, P, C)[perm].reshape(C, C)
    wpe = np.ascontiguousarray(
        wpe_cc.reshape(2, 8, P, CT, P).transpose(0, 3, 2, 1, 4)
    ).astype(NP_MMDT)

    in_maps = []
    for j in range(8):
        b, hg = j // 4, j % 4
        col0 = 2 * j * D
        in_maps.append(
            {
                "x_tb": x_tb,
                "x_res": np.ascontiguousarray(
                    x_t32[b, :, hg * TQ : (hg + 1) * TQ]
                ),
                "wq": np.ascontiguousarray(
                    w_qkv_n[:, col0 : col0 + HPC * D]
                    .reshape(CT, P, HPC, D).transpose(2, 1, 0, 3)
                ).astype(NP_MMDT),
                "wk": np.ascontiguousarray(
                    w_qkv_n[:, C + col0 : C + col0 + HPC * D]
                    .reshape(CT, P, HPC, D).transpose(2, 1, 0, 3)
                ).astype(NP_MMDT),
                "wv": np.ascontiguousarray(
                    w_qkv_n[:, 2 * C + col0 : 2 * C + col0 + HPC * D]
                    .reshape(CT, P, HPC * D).transpose(1, 0, 2)
                ).astype(NP_MMDT),
                "wpe": wpe,
                "w1t": w1t,
                "w2t": w2t,
                "w3": w3_b,
                "rope_t": rope_tab,
                "rope_sw": rope_tab_sw,
                "tri": tri,
            }
        )
    return in_maps


def kernel(x, w_norm1, w_qkv, w_proj, w_norm2, w1, w2, w3, _trace=False, _tmpdir=None):
    nc = _get_nc()
    in_maps = _host_inputs(x, w_norm1, w_qkv, w_proj, w_norm2, w1, w2, w3)
    kwargs = {}
    if _trace:
        kwargs = {"trace": True, "tmpdir": _tmpdir}
    res = bass_utils.run_bass_kernel_spmd(
        nc, in_maps, core_ids=list(range(8)), **kwargs
    )
    out = np.empty((B, T, C), dtype=np.float32)
    for j in range(8):
        out[j // 4, (j % 4) * TQ : (j % 4 + 1) * TQ, :] = res.results[j]["out_t"].T
    kernel._last_exec_time_ns = res.exec_time_ns
    return out
